# revision 2
# baseline (speedup 1.0000x reference)
"""Trainium2 Bass kernel for AttentionAggregator (GNN message passing).

Reference computation:
    new_emb = fb @ W + b
    s_e     = (fa @ a1)[src_e] + (new_emb @ a2)[dst_e]
    score_e = exp(elu(s_e, 0.1))
    out[n]  = (sum_{e: src_e=n} score_e * new_emb[dst_e]) / max(den[n], den==0->1)

Algebraic reformulation used here (linearity of the segment sum):
    q_e   = fb[dst_e] @ (W @ a2)            # per-edge scalar
    s_e   = (fa @ a1)[src_e] + q_e + b @ a2
    G[n]  = sum_e score_e * fb[dst_e]       # [Na, 64]
    den[n]= sum_e score_e
    out[n]= (G[n] / den_safe[n]) @ W + 1[den[n] > 0] * b

so new_emb is never materialized; only raw fb rows are gathered.

Distribution: nodes (and their incoming edge lists, after a host-side sort of
edges by src) are sharded contiguously across the 8 cores.  Each core owns
6272 output rows, gathers fb rows for its own edges (fb replicated), and no
cross-core collective is needed.

Device-side layout: each node's edge list is split into "virtual nodes" of at
most D0=24 slots.  A group of 128 virtual nodes occupies the 128 partitions;
their slot indices are gathered with one multi-index indirect DMA
([128, B*24] indices -> [128, B*24*64] fb rows for B groups per call).  Slot
scores are computed with per-slot ops, and the slot dimension is reduced with
strided-AP tensor_reduce.  A second tiny pass combines the <=KV virtual rows
of each real node, divides by den, and applies the final @W (+b) with the PE.
"""

import sys

for _p in ("/opt/trn_rl_repo",):
    if _p not in sys.path:
        sys.path.insert(0, _p)

import numpy as np

import concourse.bass as bass
import concourse.bacc as bacc
import concourse.mybir as mybir
import concourse.tile as tile
from concourse.bass import IndirectOffsetOnAxis
from concourse.masks import make_identity

P = 128
F = 64          # feature dim
D0 = 12         # edge slots per virtual node
NCORES = 8

f32 = mybir.dt.float32
bf16 = mybir.dt.bfloat16
i32 = mybir.dt.int32
i16 = mybir.dt.int16
TC = 16          # nodes per partition in the table-build pass
TW = 128         # augmented table row (bf16): fb (64) | q (1) | pad -> 256B
AX = mybir.AxisListType
OP = mybir.AluOpType
ACTF = mybir.ActivationFunctionType


# ----------------------------------------------------------------------------
# device program
# ----------------------------------------------------------------------------

def emit_program(tc, ins, outs, cfg):
    """Emit the per-core program.

    ins:  dict of APs: fb_tab [NB, F], pk [Gv, P, PKW], cpk [Gc, P, 2*KV],
          wvec [P, 3*F], wmat [F, F]
    outs: dict of APs: out [Gc*P, F], vtab [Gvp*P, VW] (scratch, Internal)
    cfg:  dict(Gv, Gc, KV, B, ba2)
    """
    nc = tc.nc
    Gv, Gc, KV, B = cfg["Gv"], cfg["Gc"], cfg["KV"], cfg["B"]
    ba2 = float(cfg["ba2"])
    fb_tab = ins["fb_tab"]
    pk = ins["pk"]
    cpk = ins["cpk"]
    wvec = ins["wvec"]
    wmat = ins["wmat"]
    out = outs["out"]
    vtab = outs["vtab"]
    tab2 = outs["tab2"]
    NB_pad = fb_tab.shape[0]

    gidx = ins["gidx"]
    cidx = ins["cidx"]
    PKW = F + D0              # fa_v row | slot mask
    NIa, H = cfg["NIa"], cfg["H"]
    VW = 128                  # vtab row (f32): G (64) | den (1) | pad -> 512B
    S = B * D0                # slots per phase-1 iteration
    NI = Gv // B
    assert Gv % B == 0

    with (
        tc.tile_pool(name="const", bufs=1) as cpool,
        tc.tile_pool(name="work", bufs=4) as pool,
        tc.tile_pool(name="psum", bufs=3, space="PSUM") as psum,
    ):
        wvec_t = cpool.tile([P, 3 * F], f32)
        nc.sync.dma_start(out=wvec_t[:], in_=wvec)
        wmat_t = cpool.tile([P, F], f32)
        nc.sync.dma_start(out=wmat_t[0:F, :], in_=wmat)
        nc.sync.dma_start(out=wmat_t[F:2 * F, :], in_=wmat)
        ident = cpool.tile([P, P], f32)
        make_identity(nc, ident[:])
        zbias = cpool.tile([P, 1], f32)
        nc.vector.memset(zbias[:], 0.0)
        mbias = cpool.tile([P, 1], f32)
        nc.vector.memset(mbias[:], -0.1)

        a1v = wvec_t[:, 0:F]
        w2v = wvec_t[:, F:2 * F]
        bv = wvec_t[:, 2 * F:3 * F]
        w2b = cpool.tile([P, F], bf16)
        nc.vector.tensor_copy(out=w2b[:], in_=w2v)

        # ---------------- phase 0: build augmented bf16 table [fb | q] -----
        abl = cfg.get("ablate", set())
        NTI = NB_pad // (P * TC) if "p0" not in abl else 0
        # tiles 0..NTA-1 cover table half A (rows [0, H)); phase-1 A-batches
        # only read half A, so half-B tiles can build concurrently with them.
        NTA = min(NTI, -(-H // (P * TC)))
        fb4 = fb_tab.rearrange("(j p c) f -> j p c f", p=P, c=TC)
        t24 = tab2.rearrange("(j p c) w -> j p c w", p=P, c=TC)

        def build_tile(j):
            fbb = pool.tile([P, TC * F], bf16, tag="fbb")
            fbb3 = fbb[:].rearrange("p (c f) -> p c f", f=F)
            nc.gpsimd.dma_start(out=fbb3, in_=fb4[j])  # SWDGE cast f32->bf16
            prodt = pool.tile([P, TC * F], bf16, tag="prodt")
            prodt3 = prodt[:].rearrange("p (c f) -> p c f", f=F)
            nc.vector.tensor_tensor(
                out=prodt3, in0=fbb3,
                in1=w2b[:, None, :].to_broadcast([P, TC, F]), op=OP.mult,
            )
            qt = pool.tile([P, TC], f32, tag="qt")
            nc.vector.tensor_reduce(
                out=qt[:], in_=prodt3, axis=AX.X, op=OP.add,
            )
            pck = pool.tile([P, TC * TW], bf16, tag="pck")
            pck3 = pck[:].rearrange("p (c w) -> p c w", w=TW)
            nc.vector.tensor_copy(out=pck3[:, :, F:F + 1], in_=qt[:, :, None])
            nc.vector.tensor_copy(out=pck3[:, :, 0:F], in_=fbb3)
            nc.sync.dma_start(out=t24[j][:, :, 0:F + 1], in_=pck3[:, :, 0:F + 1])

        for j in range(NTI):
            build_tile(j)
        tc.strict_bb_all_engine_barrier()

        # ---------------- phase 1: per-virtual-node-group segment sums -----
        def phase1_iter(it):
            g0 = it * B
            pk_t = pool.tile([P, B * PKW], f32, tag="pk")
            nc.sync.dma_start(
                out=pk_t[:].rearrange("p (b w) -> p b w", w=PKW),
                in_=pk[g0:g0 + B].rearrange("b p w -> p b w"),
            )
            pk3 = pk_t[:].rearrange("p (b w) -> p b w", w=PKW)
            gi_t = pool.tile([P, S * P // 16], i16, tag="gi", bufs=4)
            nc.sync.dma_start(out=gi_t[:], in_=gidx[it])

            rows = pool.tile([P, S * TW], bf16, tag="rows", bufs=4)
            rows3 = rows[:].rearrange("p (s w) -> p s w", w=TW)  # [P, S, 128]
            half = tab2[0:H, :] if it < NIa else tab2[H:2 * H, :]
            NIDX = cfg.get("nidx", 1024)      # per-call ring-capacity limit
            off = 0
            while off < S * P and "gather" not in abl:
                n = min(NIDX, S * P - off)
                nc.gpsimd.dma_gather(
                    out_ap=rows3[:, off // P:(off + n) // P, :],
                    in_ap=half,
                    idxs_ap=gi_t[:, off // 16:(off + n) // 16],
                    num_idxs=n,
                    num_idxs_reg=n,
                    elem_size=TW,
                )
                off += n
            # e1[p, b] = fa_v[p, b, :] @ a1
            fprod = pool.tile([P, B * F], f32, tag="fprod")
            nc.vector.tensor_tensor(
                out=fprod[:].rearrange("p (b f) -> p b f", f=F),
                in0=pk3[:, :, 0:F],
                in1=a1v[:, None, :].to_broadcast([P, B, F]),
                op=OP.mult,
            )
            e1 = pool.tile([P, B], f32, tag="e1")
            nc.vector.tensor_reduce(
                out=e1[:],
                in_=fprod[:].rearrange("p (b f) -> p b f", f=F),
                axis=AX.X, op=OP.add,
            )
            if ba2 != 0.0:
                nc.vector.tensor_scalar(
                    out=e1[:], in0=e1[:], scalar1=ba2, scalar2=None, op0=OP.add,
                )

            # s = q + e1; q is the gathered bf16 column 64 (+ ba2 in e1)
            s_t = pool.tile([P, S], f32, tag="s")
            nc.vector.tensor_tensor(
                out=s_t[:].rearrange("p (b k) -> p b k", k=D0),
                in0=rows3[:, :, F].rearrange("p (b k) -> p b k", k=D0),
                in1=e1[:, :, None].to_broadcast([P, B, D0]),
                op=OP.add,
            )

            # score = where(s + ba2 > 0, exp(s + ba2), exp(0.1*exp(s+ba2) - 0.1))
            t_t = pool.tile([P, S], f32, tag="t")
            nc.scalar.activation(t_t[:], s_t[:], ACTF.Exp, bias=zbias[:, 0:1],
                                 scale=1.0)
            u_t = pool.tile([P, S], f32, tag="u")
            nc.scalar.activation(u_t[:], t_t[:], ACTF.Exp, bias=mbias[:, 0:1],
                                 scale=0.1)
            m_t = pool.tile([P, S], mybir.dt.uint8, tag="m")
            nc.vector.tensor_scalar(
                out=m_t[:], in0=s_t[:], scalar1=0.0, scalar2=None, op0=OP.is_gt,
            )
            nc.vector.copy_predicated(out=u_t[:], mask=m_t[:], data=t_t[:])
            # zero padded slots and downcast to bf16 in one op
            u2 = pool.tile([P, S], bf16, tag="u2")
            nc.vector.tensor_tensor(
                out=u2[:].rearrange("p (b k) -> p b k", k=D0),
                in0=u_t[:].rearrange("p (b k) -> p b k", k=D0),
                in1=pk3[:, :, F:F + D0],
                op=OP.mult,
            )
            scaled = pool.tile([P, S * F], bf16, tag="scaled", bufs=2)
            scaled3 = scaled[:].rearrange("p (s f) -> p s f", f=F)
            vout = pool.tile([P, B * VW], f32, tag="vout")
            vout3 = vout[:].rearrange("p (b w) -> p b w", w=VW)
            if "big" not in abl:
                nc.vector.tensor_tensor(
                    out=scaled3,
                    in0=rows3[:, :, 0:F],
                    in1=u2[:, :, None].to_broadcast([P, S, F]),
                    op=OP.mult,
                )
                nc.vector.tensor_reduce(
                    out=vout3[:, :, 0:F],
                    in_=scaled[:].rearrange("p (b k f) -> p b f k", k=D0, f=F),
                    axis=AX.X, op=OP.add,
                )
            nc.vector.tensor_reduce(
                out=vout3[:, :, F:F + 1],
                in_=u2[:].rearrange("p (b k) -> p b k", k=D0),
                axis=AX.X, op=OP.add,
            )
            nc.sync.dma_start(
                out=vtab.rearrange("(g p) w -> g p w", p=P)[g0:g0 + B]
                    .rearrange("b p w -> p b w")[:, :, 0:F + 1],
                in_=vout3[:, :, 0:F + 1],
            )

        for it in range(NI):
            phase1_iter(it)

        # ---------------- phase 2: combine virtual rows, divide, @W + b ----
        B2 = cfg["B2"]
        Gc2 = cfg["Gc2"]
        out3 = out.rearrange("(g p) f -> g p f", p=P)
        for r2 in range(Gc2 // B2 if cfg.get("phases", "all") == "all" else 0):
            r0 = r2 * B2
            cpk_t = pool.tile([P, B2 * KV], f32, tag="cpk")
            nc.sync.dma_start(
                out=cpk_t[:].rearrange("p (b k) -> p b k", k=KV),
                in_=cpk[r0:r0 + B2].rearrange("b p k -> p b k"),
            )
            cm = cpk_t[:, 0:B2 * KV]
            ci_t = pool.tile([P, B2 * KV * P // 16], i16, tag="ci")
            nc.sync.dma_start(out=ci_t[:], in_=cidx[r2])

            gr = pool.tile([P, B2 * KV * VW], f32, tag="gr")
            gr3 = gr[:].rearrange("p (k w) -> p k w", w=VW)   # [P, B2*KV, VW]
            nc.gpsimd.dma_gather(
                out_ap=gr3,
                in_ap=vtab,
                idxs_ap=ci_t[:],
                num_idxs=B2 * KV * P,
                num_idxs_reg=B2 * KV * P,
                elem_size=VW,
            )

            scm = pool.tile([P, B2 * KV * (F + 1)], f32, tag="scm")
            nc.vector.tensor_tensor(
                out=scm[:].rearrange("p (k w) -> p k w", w=F + 1),
                in0=gr3[:, :, 0:F + 1],
                in1=cm[:, :, None].to_broadcast([P, B2 * KV, F + 1]),
                op=OP.mult,
            )
            hd = pool.tile([P, B2 * (F + 1)], f32, tag="hd")
            hd3 = hd[:].rearrange("p (b w) -> p b w", w=F + 1)
            nc.vector.tensor_reduce(
                out=hd3,
                in_=scm[:].rearrange("p (b k w) -> p b w k", k=KV, w=F + 1),
                axis=AX.X, op=OP.add,
            )
            den = hd3[:, :, F]                                 # [P, B2]
            m0 = pool.tile([P, B2], f32, tag="m0")
            nc.vector.tensor_scalar(
                out=m0[:], in0=den, scalar1=0.0, scalar2=None, op0=OP.is_equal,
            )
            dsafe = pool.tile([P, B2], f32, tag="dsafe")
            nc.vector.tensor_tensor(out=dsafe[:], in0=den, in1=m0[:], op=OP.add)
            rec = pool.tile([P, B2], f32, tag="rec")
            nc.vector.reciprocal(rec[:], dsafe[:])
            h_t = pool.tile([P, B2 * F], f32, tag="h")
            nc.vector.tensor_tensor(
                out=h_t[:].rearrange("p (b f) -> p b f", f=F),
                in0=hd3[:, :, 0:F],
                in1=rec[:, :, None].to_broadcast([P, B2, F]),
                op=OP.mult,
            )
            w1 = pool.tile([P, B2], f32, tag="w1")
            nc.vector.tensor_scalar(
                out=w1[:], in0=den, scalar1=0.0, scalar2=None, op0=OP.is_gt,
            )
            outs_t = pool.tile([P, B2 * F], f32, tag="outs")
            for b0 in range(0, B2, 2):
                bw = min(2, B2 - b0)
                htp = psum.tile([bw * F, P], f32, tag="htp")
                nc.tensor.transpose(
                    out=htp[:], in_=h_t[:, b0 * F:(b0 + bw) * F],
                    identity=ident[:])
                ht = pool.tile([bw * F, P], f32, tag="ht")
                nc.vector.tensor_copy(out=ht[:], in_=htp[:])
                for bb in range(bw):
                    b = b0 + bb
                    op_t = psum.tile([P, F], f32, tag="op")
                    nc.tensor.matmul(out=op_t[:],
                                     lhsT=ht[bb * F:(bb + 1) * F, :],
                                     rhs=wmat_t[bb * F:(bb + 1) * F, :],
                                     start=True, stop=True)
                    badd = pool.tile([P, F], f32, tag="badd")
                    nc.vector.tensor_scalar(
                        out=badd[:], in0=bv, scalar1=w1[:, b:b + 1],
                        scalar2=None, op0=OP.mult,
                    )
                    nc.vector.tensor_tensor(
                        out=outs_t[:, b * F:(b + 1) * F], in0=op_t[:],
                        in1=badd[:], op=OP.add,
                    )
            nc.sync.dma_start(
                out=out3[r0:r0 + B2].rearrange("g p f -> p g f"),
                in_=outs_t[:].rearrange("p (b f) -> p b f", f=F),
            )


# ----------------------------------------------------------------------------
# host-side preparation
# ----------------------------------------------------------------------------

def prep_inputs(feature_a, feature_b, W, b, a_vec, edges, node_num_a,
                ncores=NCORES, d0=D0):
    """Shard + pad inputs for the SPMD program.  Index plumbing only (sort,
    bincount, padding); the only host arithmetic is the tiny parameter
    derivation Wa2 = W @ a2 (64x64 matvec) and ba2 = b @ a2."""
    fa = np.asarray(feature_a, np.float32)
    fb = np.asarray(feature_b, np.float32)
    W = np.asarray(W, np.float32)
    b = np.asarray(b, np.float32)
    a_vec = np.asarray(a_vec, np.float32).reshape(-1)
    edges = np.asarray(edges)
    NA = int(node_num_a)
    NB, Fdim = fb.shape
    assert Fdim == F and fa.shape[1] == F

    src = edges[:, 0].astype(np.int64)
    dst = edges[:, 1].astype(np.int64)

    NB_pad = -(-NB // (P * TC)) * (P * TC)
    fb_pad = np.zeros((NB_pad, F), np.float32)
    fb_pad[:NB] = fb

    a1 = a_vec[:F]
    a2 = a_vec[F:]
    Wa2 = (W @ a2).astype(np.float32)
    ba2 = float(b @ a2)

    H = NB_pad // 2
    hflag = (dst >= H).astype(np.int64)
    order = np.lexsort((hflag, src))
    ssrc = src[order]
    sdst = dst[order].astype(np.int64)
    shf = hflag[order]
    deg = np.bincount(ssrc, minlength=NA).astype(np.int64)
    degA = np.bincount(ssrc[shf == 0], minlength=NA).astype(np.int64)
    degB = deg - degA
    row_ptr = np.zeros(NA + 1, np.int64)
    np.cumsum(deg, out=row_ptr[1:])

    nodes_per_core = -(-NA // (ncores * P)) * P          # 6272
    Gc = nodes_per_core // P                             # 49
    nvA = -(-degA // d0)
    nvB = -(-degB // d0)
    KV = max(2, int((nvA + nvB).max()))
    B = 4

    def build_half(lo, hi, degH, nvH, edge_off):
        """Virtual nodes for one dst-half of one core's node range.
        edge_off[n] = first sorted-edge position of this half's run."""
        n_nodes = max(hi - lo, 0)
        node_ids = np.arange(lo, hi)
        nvc = nvH[lo:hi] if n_nodes else np.zeros(0, np.int64)
        Nv = int(nvc.sum())
        vnode = np.repeat(node_ids, nvc)
        vstart0 = np.concatenate([[0], np.cumsum(nvc)])[:-1]
        vrank = np.arange(Nv) - np.repeat(vstart0, nvc)
        pos = edge_off[vnode][:, None] + vrank[:, None] * d0 + np.arange(d0)[None, :]
        valid = (vrank[:, None] * d0 + np.arange(d0)[None, :]) < degH[vnode][:, None]
        posc = np.clip(pos, 0, max(len(sdst) - 1, 0))
        sidx = np.where(valid, sdst[posc] if len(sdst) else 0, 0).astype(np.int64)
        return dict(Nv=Nv, vnode=vnode, nvc=nvc, vstart0=vstart0,
                    sidx=sidx, valid=valid)

    offA = row_ptr[:-1]            # A-run starts at the node's run start
    offB = row_ptr[:-1] + degA     # B-run follows
    cores = []
    for c in range(ncores):
        lo = c * nodes_per_core
        hi = min(lo + nodes_per_core, NA)
        ha = build_half(lo, hi, degA, nvA, offA)
        hb = build_half(lo, hi, degB, nvB, offB)
        hb["sidx"] = np.where(hb["valid"], hb["sidx"] - H, 0)
        cores.append((ha, hb))

    def cdiv(a, b):
        return -(-a // b)

    maxA = max(1, max(h[0]["Nv"] for h in cores))
    maxB = max(h[1]["Nv"] for h in cores)
    GvA = cdiv(cdiv(maxA, P), B) * B
    GvB = cdiv(cdiv(maxB, P), B) * B if maxB > 0 else 0
    NIa = GvA // B
    Gv = GvA + GvB
    Nvp = Gv * P
    B2 = min(4, max(1, 1024 // (KV * P)))
    Gc2 = cdiv(Gc, B2) * B2

    in_maps = []
    PKW = F + d0
    S = B * d0
    NI = Gv // B
    for c in range(ncores):
        ha, hb = cores[c]
        pk = np.zeros((Nvp, PKW), np.float32)
        sidx_all = np.zeros((Nvp, d0), np.int64)
        for (h, base) in ((ha, 0), (hb, GvA * P)):
            Nv = h["Nv"]
            if Nv:
                pk[base:base + Nv, 0:F] = fa[h["vnode"]]
                pk[base:base + Nv, F:F + d0] = h["valid"].astype(np.float32)
                sidx_all[base:base + Nv] = h["sidx"]
        pk = pk.reshape(Gv, P, PKW)

        # int16 gather indices: per batch, flat[(b*d0+k)*128 + p] =
        # sidx[group g0+b, partition p, slot k]; sbuf wrap [16, S*128//16],
        # replicated to 128 partitions; stored bitcast-f32 inside pk so one
        # DMA loads fa_v + mask + idx.  Device reads pk3[:, :, F+D0:] as the
        # per-iteration [P, S*P//16] i16 block, so per-group cols must hold
        # that group's quarter of the batch block: columns [b*d0*8*(..)].
        sidx_g = sidx_all.reshape(Gv, P, d0)
        gidx16 = np.zeros((NI, P, S * P // 16), np.int16)
        for i in range(NI):
            blk = sidx_g[i * B:(i + 1) * B]              # [B, P, d0]
            flat = blk.transpose(0, 2, 1).reshape(-1)    # [(b k) p]
            sb = flat.reshape(S * P // 16, 16).T.astype(np.int16)
            gidx16[i] = np.tile(sb, (8, 1))
        assert sidx_all.max() < 32768

        cpka = np.zeros((Gc2 * P, KV), np.float32)
        cidxa = np.zeros((Gc2 * P, KV), np.int64)
        n_nodes = min(nodes_per_core, NA - c * nodes_per_core)
        if n_nodes > 0:
            nv_tot = ha["nvc"] + hb["nvc"]
            ks = np.arange(KV)[None, :]
            cvalid = ks < nv_tot[:, None]
            # first the node's A-virtual rows, then its B-virtual rows
            inA = ks < ha["nvc"][:, None]
            idxA = ha["vstart0"][:, None] + ks
            idxB = GvA * P + hb["vstart0"][:, None] + (ks - ha["nvc"][:, None])
            cidxv = np.where(cvalid, np.where(inA, idxA, idxB), 0)
            cpka[:n_nodes, 0:KV] = cvalid.astype(np.float32)
            cidxa[:n_nodes] = cidxv
        cpk = cpka.reshape(Gc2, P, KV)
        assert cidxa.max() < 32768
        # int16 wrap for phase-2 dma_gather, one batch of B2 groups per call:
        # flat[(b*KV + k)*128 + p] = cidx[group r0+b, p, k]
        cg = cidxa.reshape(Gc2, P, KV)
        NW = B2 * KV * P // 16
        cidx16 = np.zeros((Gc2 // B2, P, NW), np.int16)
        for r in range(Gc2 // B2):
            flat = cg[r * B2:(r + 1) * B2].transpose(0, 2, 1).reshape(-1)
            sb = flat.reshape(NW, 16).T.astype(np.int16)
            cidx16[r] = np.tile(sb, (8, 1))

        wvec = np.zeros((P, 3 * F), np.float32)
        wvec[:, 0:F] = a1[None, :]
        wvec[:, F:2 * F] = Wa2[None, :]
        wvec[:, 2 * F:3 * F] = b[None, :]

        in_maps.append(dict(
            fb_tab=fb_pad,
            pk=np.ascontiguousarray(pk),
            gidx=np.ascontiguousarray(gidx16),
            cpk=np.ascontiguousarray(cpk),
            cidx=np.ascontiguousarray(cidx16),
            wvec=wvec,
            wmat=np.ascontiguousarray(W),
        ))

    cfg = dict(Gv=Gv, Gc=Gc, Gc2=Gc2, B2=B2, KV=KV, B=B, ba2=ba2, NB=NB,
               NB_pad=NB_pad, NIa=NIa, H=H, Nvp=Nvp,
               nodes_per_core=nodes_per_core, NA=NA)
    return in_maps, cfg


def build_bass(cfg, ncores=NCORES):
    nc = bacc.Bacc("TRN2", target_bir_lowering=False, debug=False,
                   enable_asserts=False, num_devices=ncores)
    ins = dict(
        fb_tab=nc.dram_tensor("fb_tab", [cfg["NB_pad"], F], f32,
                              kind="ExternalInput").ap(),
        pk=nc.dram_tensor("pk", [cfg["Gv"], P, F + D0], f32,
                          kind="ExternalInput").ap(),
        gidx=nc.dram_tensor("gidx", [cfg["Gv"] // cfg["B"], P,
                                     cfg["B"] * D0 * P // 16], i16,
                            kind="ExternalInput").ap(),
        cpk=nc.dram_tensor("cpk", [cfg["Gc2"], P, cfg["KV"]], f32,
                           kind="ExternalInput").ap(),
        cidx=nc.dram_tensor("cidx", [cfg["Gc2"] // cfg["B2"], P,
                                     cfg["B2"] * cfg["KV"] * P // 16], i16,
                            kind="ExternalInput").ap(),
        wvec=nc.dram_tensor("wvec", [P, 3 * F], f32, kind="ExternalInput").ap(),
        wmat=nc.dram_tensor("wmat", [F, F], f32, kind="ExternalInput").ap(),
    )
    outs = dict(
        out=nc.dram_tensor("out", [cfg["Gc2"] * P, F], f32,
                           kind="ExternalOutput").ap(),
        vtab=nc.dram_tensor("vtab", [cfg["Nvp"], 128], f32,
                            kind="ExternalOutput").ap(),
        tab2=nc.dram_tensor("tab2", [cfg["NB_pad"], TW], bf16,
                            kind="ExternalOutput").ap(),
    )
    with tile.TileContext(nc) as tc:
        emit_program(tc, ins, outs, cfg)
    nc.compile()
    return nc


# ----------------------------------------------------------------------------
# entry point
# ----------------------------------------------------------------------------

def kernel_with_results(trace=False, **inputs):
    from concourse import bass_utils

    in_maps, cfg = prep_inputs(**inputs)
    nc = build_bass(cfg)
    res = bass_utils.run_bass_kernel_spmd(
        nc, in_maps, core_ids=list(range(NCORES)), trace=trace,
    )
    outs = [r["out"][:cfg["nodes_per_core"]] for r in res.results]
    full = np.concatenate(outs, axis=0)[:cfg["NA"]]
    return full.astype(np.float32), res


def kernel(**inputs):
    return kernel_with_results(trace=False, **inputs)[0]


def kernel_timed(nreps=6, **inputs):
    """Like kernel(), but reuses the compiled PJRT executable and times warm
    repeat executions with device-resident inputs.  Returns (out, [ns,...])."""
    import time
    import jax
    from jax.sharding import Mesh, PartitionSpec, NamedSharding
    from jax.experimental.shard_map import shard_map
    from concourse import bass2jax

    in_maps, cfg = prep_inputs(**inputs)
    nc = build_bass(cfg)
    bass2jax.install_neuronx_cc_hook()

    ncores = NCORES
    partition_name = nc.partition_id_tensor.name if nc.partition_id_tensor else None
    in_names, out_names, out_avals, zero_outs = [], [], [], []
    for alloc in nc.m.functions[0].allocations:
        if not isinstance(alloc, mybir.MemoryLocationSet):
            continue
        name = alloc.memorylocations[0].name
        if alloc.kind == "ExternalInput":
            if name != partition_name:
                in_names.append(name)
        elif alloc.kind == "ExternalOutput":
            shape = tuple(alloc.tensor_shape)
            dtype = mybir.dt.np(alloc.dtype)
            out_avals.append(jax.core.ShapedArray(shape, dtype))
            out_names.append(name)
            zero_outs.append(np.zeros(shape, dtype))
    n_params = len(in_names)
    n_outs = len(out_avals)
    all_in_names = list(in_names) + list(out_names)
    if partition_name is not None:
        all_in_names.append(partition_name)

    def _body(*args):
        operands = list(args)
        if partition_name is not None:
            operands.append(bass2jax.partition_id_tensor())
        outs = bass2jax._bass_exec_p.bind(
            *operands,
            out_avals=tuple(out_avals),
            in_names=tuple(all_in_names),
            out_names=tuple(out_names),
            lowering_input_output_aliases=(),
            sim_require_finite=True,
            sim_require_nnan=True,
            nc=nc,
        )
        return tuple(outs)

    devices = jax.devices()[:ncores]
    mesh = Mesh(np.asarray(devices), ("core",))
    spec = PartitionSpec("core")
    shard = NamedSharding(mesh, spec)
    donate = tuple(range(n_params, n_params + n_outs))
    sharded = jax.jit(
        shard_map(_body, mesh=mesh, in_specs=(spec,) * (n_params + n_outs),
                  out_specs=(spec,) * n_outs, check_rep=False),
        donate_argnums=donate, keep_unused=True,
    )
    concat_in = [
        np.concatenate([np.asarray(in_maps[c][nm]) for c in range(ncores)], axis=0)
        for nm in in_names
    ]
    concat_zeros = [
        np.zeros((ncores * z.shape[0], *z.shape[1:]), z.dtype) for z in zero_outs
    ]
    dev_in = [jax.device_put(a, shard) for a in concat_in]

    out_arrs = None
    times = []
    for rep in range(nreps + 1):
        dz = [jax.device_put(z, shard) for z in concat_zeros]
        for d in dz:
            d.block_until_ready()
        t0 = time.perf_counter()
        res = sharded(*dev_in, *dz)
        for r in res:
            r.block_until_ready()
        t1 = time.perf_counter()
        if rep > 0:
            times.append(int((t1 - t0) * 1e9))
        out_arrs = res

    outs = {}
    for i, name in enumerate(out_names):
        outs[name] = np.asarray(out_arrs[i]).reshape(
            ncores, *out_avals[i].shape)
    full = np.concatenate(
        [outs["out"][c][:cfg["nodes_per_core"]] for c in range(ncores)],
        axis=0)[:cfg["NA"]]
    return full.astype(np.float32), times


if __name__ == "__main__":
    np.random.seed(0)
    NA = NB = 50000
    E = 800000
    ins = dict(
        feature_a=np.random.randn(NA, F).astype(np.float32),
        feature_b=np.random.randn(NB, F).astype(np.float32),
        W=(np.random.randn(F, F) / 8).astype(np.float32),
        b=np.zeros(F, np.float32),
        a_vec=(np.random.randn(2 * F, 1) * 0.05).astype(np.float32),
        edges=np.stack([np.random.randint(0, NA, E),
                        np.random.randint(0, NB, E)], 1).astype(np.int64),
        node_num_a=NA,
    )
    out = kernel(**ins)
    print(out.shape, out.dtype)



# revision 9
# speedup vs baseline: 2.8797x; 2.8797x over previous
"""Trainium2 Bass kernel for AttentionAggregator (GNN message passing).

Reference computation:
    new_emb = fb @ W + b
    s_e     = (fa @ a1)[src_e] + (new_emb @ a2)[dst_e]
    score_e = exp(elu(s_e, 0.1))
    out[n]  = (sum_{e: src_e=n} score_e * new_emb[dst_e]) / max(den[n], den==0->1)

Algebraic reformulation used here (linearity of the segment sum):
    q_e   = fb[dst_e] @ (W @ a2)            # per-edge scalar
    s_e   = (fa @ a1)[src_e] + q_e + b @ a2
    G[n]  = sum_e score_e * fb[dst_e]       # [Na, 64]
    den[n]= sum_e score_e
    out[n]= (G[n] / den_safe[n]) @ W + 1[den[n] > 0] * b

so new_emb is never materialized; only raw fb rows are gathered.

Distribution: nodes (and their incoming edge lists, after a host-side sort of
edges by src) are sharded contiguously across the 8 cores.  Each core owns
6272 output rows, gathers fb rows for its own edges (fb replicated), and no
cross-core collective is needed.

Device-side layout: each node's edge list is split into "virtual nodes" of at
most D0=24 slots.  A group of 128 virtual nodes occupies the 128 partitions;
their slot indices are gathered with one multi-index indirect DMA
([128, B*24] indices -> [128, B*24*64] fb rows for B groups per call).  Slot
scores are computed with per-slot ops, and the slot dimension is reduced with
strided-AP tensor_reduce.  A second tiny pass combines the <=KV virtual rows
of each real node, divides by den, and applies the final @W (+b) with the PE.
"""

import sys

for _p in ("/opt/trn_rl_repo",):
    if _p not in sys.path:
        sys.path.insert(0, _p)

import numpy as np

import concourse.bass as bass
import concourse.bacc as bacc
import concourse.mybir as mybir
import concourse.tile as tile
from concourse.bass import IndirectOffsetOnAxis
from concourse.masks import make_identity

P = 128
F = 64          # feature dim
D0 = 12         # edge slots per virtual node
NCORES = 8

f32 = mybir.dt.float32
bf16 = mybir.dt.bfloat16
i32 = mybir.dt.int32
i16 = mybir.dt.int16
TC = 16          # nodes per partition in the table-build pass
TW = 128         # augmented table row (bf16): fb (64) | q (1) | pad -> 256B
AX = mybir.AxisListType
OP = mybir.AluOpType
ACTF = mybir.ActivationFunctionType


# ----------------------------------------------------------------------------
# device program
# ----------------------------------------------------------------------------

def emit_program(tc, ins, outs, cfg):
    """Emit the per-core program.

    ins:  dict of APs: fb_tab [NB, F], pk [Gv, P, PKW], cpk [Gc, P, 2*KV],
          wvec [P, 3*F], wmat [F, F]
    outs: dict of APs: out [Gc*P, F], vtab [Gvp*P, VW] (scratch, Internal)
    cfg:  dict(Gv, Gc, KV, B, ba2)
    """
    nc = tc.nc
    Gv, Gc, KV, B = cfg["Gv"], cfg["Gc"], cfg["KV"], cfg["B"]
    ba2 = float(cfg["ba2"])
    fb_tab = ins["fb_tab"]
    pk = ins["pk"]
    cpk = ins["cpk"]
    wvec = ins["wvec"]
    wmat = ins["wmat"]
    out = outs["out"]
    vtab = outs["vtab"]
    tab2 = outs["tab2"]
    NB_pad = fb_tab.shape[0]

    gidx = ins["gidx"]
    cidx = ins["cidx"]
    PKW = F + D0              # fa_v row | slot mask
    NIa, H = cfg["NIa"], cfg["H"]
    VW = 128                  # vtab row (f32): G (64) | den (1) | pad -> 512B
    S = B * D0                # slots per phase-1 iteration
    NI = Gv // B
    assert Gv % B == 0

    with (
        tc.tile_pool(name="const", bufs=1) as cpool,
        tc.tile_pool(name="work", bufs=4) as pool,
        tc.tile_pool(name="psum", bufs=3, space="PSUM") as psum,
    ):
        wvec_t = cpool.tile([P, 3 * F], f32)
        nc.sync.dma_start(out=wvec_t[:], in_=wvec)
        wmat_t = cpool.tile([P, F], f32)
        nc.sync.dma_start(out=wmat_t[0:F, :], in_=wmat)
        nc.sync.dma_start(out=wmat_t[F:2 * F, :], in_=wmat)
        ident = cpool.tile([P, P], f32)
        make_identity(nc, ident[:])
        zbias = cpool.tile([P, 1], f32)
        nc.vector.memset(zbias[:], 0.0)
        mbias = cpool.tile([P, 1], f32)
        nc.vector.memset(mbias[:], -0.1)

        a1v = wvec_t[:, 0:F]
        w2v = wvec_t[:, F:2 * F]
        bv = wvec_t[:, 2 * F:3 * F]
        w2b = cpool.tile([P, F], bf16)
        nc.vector.tensor_copy(out=w2b[:], in_=w2v)

        # ---------------- phase 0: build augmented bf16 table [fb | q] -----
        abl = cfg.get("ablate", set())
        NTI = NB_pad // (P * TC) if "p0" not in abl else 0
        # tiles 0..NTA-1 cover table half A (rows [0, H)); phase-1 A-batches
        # only read half A, so half-B tiles can build concurrently with them.
        NTA = min(NTI, -(-H // (P * TC)))
        fb4 = fb_tab.rearrange("(j p c) f -> j p c f", p=P, c=TC)
        t24 = tab2.rearrange("(j p c) w -> j p c w", p=P, c=TC)

        def build_tile(j):
            fbb = pool.tile([P, TC * F], bf16, tag="fbb")
            fbb3 = fbb[:].rearrange("p (c f) -> p c f", f=F)
            nc.gpsimd.dma_start(out=fbb3, in_=fb4[j])  # SWDGE cast f32->bf16
            prodt = pool.tile([P, TC * F], bf16, tag="prodt")
            prodt3 = prodt[:].rearrange("p (c f) -> p c f", f=F)
            nc.vector.tensor_tensor(
                out=prodt3, in0=fbb3,
                in1=w2b[:, None, :].to_broadcast([P, TC, F]), op=OP.mult,
            )
            qt = pool.tile([P, TC], f32, tag="qt")
            nc.vector.tensor_reduce(
                out=qt[:], in_=prodt3, axis=AX.X, op=OP.add,
            )
            pck = pool.tile([P, TC * TW], bf16, tag="pck")
            pck3 = pck[:].rearrange("p (c w) -> p c w", w=TW)
            nc.vector.tensor_copy(out=pck3[:, :, F:F + 1], in_=qt[:, :, None])
            nc.vector.tensor_copy(out=pck3[:, :, 0:F], in_=fbb3)
            nc.sync.dma_start(out=t24[j][:, :, 0:F + 1], in_=pck3[:, :, 0:F + 1])

        for _r in range(cfg.get("rep0", 1)):
            for j in range(NTI):
                build_tile(j)
        tc.strict_bb_all_engine_barrier()

        # ---------------- phase 1: per-virtual-node-group segment sums -----
        def phase1_iter(it):
            g0 = it * B
            pk_t = pool.tile([P, B * PKW], f32, tag="pk")
            nc.sync.dma_start(
                out=pk_t[:].rearrange("p (b w) -> p b w", w=PKW),
                in_=pk[g0:g0 + B].rearrange("b p w -> p b w"),
            )
            pk3 = pk_t[:].rearrange("p (b w) -> p b w", w=PKW)
            gi_t = pool.tile([P, S * P // 16], i16, tag="gi", bufs=4)
            nc.sync.dma_start(out=gi_t[:], in_=gidx[it])

            rows = pool.tile([P, S * TW], bf16, tag="rows", bufs=4)
            rows3 = rows[:].rearrange("p (s w) -> p s w", w=TW)  # [P, S, 128]
            half = tab2[0:H, :] if it < NIa else tab2[H:2 * H, :]
            NIDX = cfg.get("nidx", 1024)      # per-call ring-capacity limit
            off = 0
            while off < S * P and "gather" not in abl:
                n = min(NIDX, S * P - off)
                nc.gpsimd.dma_gather(
                    out_ap=rows3[:, off // P:(off + n) // P, :],
                    in_ap=half,
                    idxs_ap=gi_t[:, off // 16:(off + n) // 16],
                    num_idxs=n,
                    num_idxs_reg=n,
                    elem_size=TW,
                )
                off += n
            # e1[p, b] = fa_v[p, b, :] @ a1
            fprod = pool.tile([P, B * F], f32, tag="fprod")
            nc.vector.tensor_tensor(
                out=fprod[:].rearrange("p (b f) -> p b f", f=F),
                in0=pk3[:, :, 0:F],
                in1=a1v[:, None, :].to_broadcast([P, B, F]),
                op=OP.mult,
            )
            e1 = pool.tile([P, B], f32, tag="e1")
            nc.vector.tensor_reduce(
                out=e1[:],
                in_=fprod[:].rearrange("p (b f) -> p b f", f=F),
                axis=AX.X, op=OP.add,
            )
            if ba2 != 0.0:
                nc.vector.tensor_scalar(
                    out=e1[:], in0=e1[:], scalar1=ba2, scalar2=None, op0=OP.add,
                )

            # s = q + e1; q is the gathered bf16 column 64 (+ ba2 in e1)
            s_t = pool.tile([P, S], f32, tag="s")
            q_in = (rows3[:, :, F].rearrange("p (b k) -> p b k", k=D0)
                    if "gather" not in abl
                    else e1[:, :, None].to_broadcast([P, B, D0]))
            nc.vector.tensor_tensor(
                out=s_t[:].rearrange("p (b k) -> p b k", k=D0),
                in0=q_in,
                in1=e1[:, :, None].to_broadcast([P, B, D0]),
                op=OP.add,
            )

            # score = where(s + ba2 > 0, exp(s + ba2), exp(0.1*exp(s+ba2) - 0.1))
            t_t = pool.tile([P, S], f32, tag="t")
            nc.scalar.activation(t_t[:], s_t[:], ACTF.Exp, bias=zbias[:, 0:1],
                                 scale=1.0)
            u_t = pool.tile([P, S], f32, tag="u")
            nc.scalar.activation(u_t[:], t_t[:], ACTF.Exp, bias=mbias[:, 0:1],
                                 scale=0.1)
            m_t = pool.tile([P, S], mybir.dt.uint8, tag="m")
            nc.vector.tensor_scalar(
                out=m_t[:], in0=s_t[:], scalar1=0.0, scalar2=None, op0=OP.is_gt,
            )
            nc.vector.copy_predicated(out=u_t[:], mask=m_t[:], data=t_t[:])
            # zero padded slots and downcast to bf16 in one op
            u2 = pool.tile([P, S], bf16, tag="u2")
            nc.vector.tensor_tensor(
                out=u2[:].rearrange("p (b k) -> p b k", k=D0),
                in0=u_t[:].rearrange("p (b k) -> p b k", k=D0),
                in1=pk3[:, :, F:F + D0],
                op=OP.mult,
            )
            scaled = pool.tile([P, S * F], bf16, tag="scaled", bufs=2)
            scaled3 = scaled[:].rearrange("p (s f) -> p s f", f=F)
            vout = pool.tile([P, B * VW], f32, tag="vout")
            vout3 = vout[:].rearrange("p (b w) -> p b w", w=VW)
            if "big" not in abl:
                big_in = (rows3[:, :, 0:F] if "gather" not in abl
                          else u2[:, :, None].to_broadcast([P, S, F]))
                nc.vector.tensor_tensor(
                    out=scaled3,
                    in0=big_in,
                    in1=u2[:, :, None].to_broadcast([P, S, F]),
                    op=OP.mult,
                )
                nc.vector.tensor_reduce(
                    out=vout3[:, :, 0:F],
                    in_=scaled[:].rearrange("p (b k f) -> p b f k", k=D0, f=F),
                    axis=AX.X, op=OP.add,
                )
            nc.vector.tensor_reduce(
                out=vout3[:, :, F:F + 1],
                in_=u2[:].rearrange("p (b k) -> p b k", k=D0),
                axis=AX.X, op=OP.add,
            )
            nc.sync.dma_start(
                out=vtab.rearrange("(g p) w -> g p w", p=P)[g0:g0 + B]
                    .rearrange("b p w -> p b w")[:, :, 0:F + 1],
                in_=vout3[:, :, 0:F + 1],
            )

        for _r in range(cfg.get("rep1", 1)):
            for it in range(NI):
                phase1_iter(it)

        # ---------------- phase 2: combine virtual rows, divide, @W + b ----
        B2 = cfg["B2"]
        Gc2 = cfg["Gc2"]
        out3 = out.rearrange("(g p) f -> g p f", p=P)
        n_r2 = Gc2 // B2 if cfg.get("phases", "all") == "all" else 0
        for r2 in [r for _ in range(cfg.get("rep2", 1)) for r in range(n_r2)]:
            r0 = r2 * B2
            cpk_t = pool.tile([P, B2 * KV], f32, tag="cpk")
            nc.sync.dma_start(
                out=cpk_t[:].rearrange("p (b k) -> p b k", k=KV),
                in_=cpk[r0:r0 + B2].rearrange("b p k -> p b k"),
            )
            cm = cpk_t[:, 0:B2 * KV]
            ci_t = pool.tile([P, B2 * KV * P // 16], i16, tag="ci")
            nc.sync.dma_start(out=ci_t[:], in_=cidx[r2])

            gr = pool.tile([P, B2 * KV * VW], f32, tag="gr")
            gr3 = gr[:].rearrange("p (k w) -> p k w", w=VW)   # [P, B2*KV, VW]
            nc.gpsimd.dma_gather(
                out_ap=gr3,
                in_ap=vtab,
                idxs_ap=ci_t[:],
                num_idxs=B2 * KV * P,
                num_idxs_reg=B2 * KV * P,
                elem_size=VW,
            )

            scm = pool.tile([P, B2 * KV * (F + 1)], f32, tag="scm")
            nc.vector.tensor_tensor(
                out=scm[:].rearrange("p (k w) -> p k w", w=F + 1),
                in0=gr3[:, :, 0:F + 1],
                in1=cm[:, :, None].to_broadcast([P, B2 * KV, F + 1]),
                op=OP.mult,
            )
            hd = pool.tile([P, B2 * (F + 1)], f32, tag="hd")
            hd3 = hd[:].rearrange("p (b w) -> p b w", w=F + 1)
            nc.vector.tensor_reduce(
                out=hd3,
                in_=scm[:].rearrange("p (b k w) -> p b w k", k=KV, w=F + 1),
                axis=AX.X, op=OP.add,
            )
            den = hd3[:, :, F]                                 # [P, B2]
            m0 = pool.tile([P, B2], f32, tag="m0")
            nc.vector.tensor_scalar(
                out=m0[:], in0=den, scalar1=0.0, scalar2=None, op0=OP.is_equal,
            )
            dsafe = pool.tile([P, B2], f32, tag="dsafe")
            nc.vector.tensor_tensor(out=dsafe[:], in0=den, in1=m0[:], op=OP.add)
            rec = pool.tile([P, B2], f32, tag="rec")
            nc.vector.reciprocal(rec[:], dsafe[:])
            h_t = pool.tile([P, B2 * F], f32, tag="h")
            nc.vector.tensor_tensor(
                out=h_t[:].rearrange("p (b f) -> p b f", f=F),
                in0=hd3[:, :, 0:F],
                in1=rec[:, :, None].to_broadcast([P, B2, F]),
                op=OP.mult,
            )
            w1 = pool.tile([P, B2], f32, tag="w1")
            nc.vector.tensor_scalar(
                out=w1[:], in0=den, scalar1=0.0, scalar2=None, op0=OP.is_gt,
            )
            outs_t = pool.tile([P, B2 * F], f32, tag="outs")
            for b0 in range(0, B2, 2):
                bw = min(2, B2 - b0)
                htp = psum.tile([bw * F, P], f32, tag="htp")
                nc.tensor.transpose(
                    out=htp[:], in_=h_t[:, b0 * F:(b0 + bw) * F],
                    identity=ident[:])
                ht = pool.tile([bw * F, P], f32, tag="ht")
                nc.vector.tensor_copy(out=ht[:], in_=htp[:])
                for bb in range(bw):
                    b = b0 + bb
                    op_t = psum.tile([P, F], f32, tag="op")
                    nc.tensor.matmul(out=op_t[:],
                                     lhsT=ht[bb * F:(bb + 1) * F, :],
                                     rhs=wmat_t[bb * F:(bb + 1) * F, :],
                                     start=True, stop=True)
                    badd = pool.tile([P, F], f32, tag="badd")
                    nc.vector.tensor_scalar(
                        out=badd[:], in0=bv, scalar1=w1[:, b:b + 1],
                        scalar2=None, op0=OP.mult,
                    )
                    nc.vector.tensor_tensor(
                        out=outs_t[:, b * F:(b + 1) * F], in0=op_t[:],
                        in1=badd[:], op=OP.add,
                    )
            nc.sync.dma_start(
                out=out3[r0:r0 + B2].rearrange("g p f -> p g f"),
                in_=outs_t[:].rearrange("p (b f) -> p b f", f=F),
            )


# ----------------------------------------------------------------------------
# host-side preparation
# ----------------------------------------------------------------------------

def prep_inputs(feature_a, feature_b, W, b, a_vec, edges, node_num_a,
                ncores=NCORES, d0=D0):
    """Shard + pad inputs for the SPMD program.  Index plumbing only (sort,
    bincount, padding); the only host arithmetic is the tiny parameter
    derivation Wa2 = W @ a2 (64x64 matvec) and ba2 = b @ a2."""
    fa = np.asarray(feature_a, np.float32)
    fb = np.asarray(feature_b, np.float32)
    W = np.asarray(W, np.float32)
    b = np.asarray(b, np.float32)
    a_vec = np.asarray(a_vec, np.float32).reshape(-1)
    edges = np.asarray(edges)
    NA = int(node_num_a)
    NB, Fdim = fb.shape
    assert Fdim == F and fa.shape[1] == F

    src = edges[:, 0].astype(np.int64)
    dst = edges[:, 1].astype(np.int64)

    NB_pad = -(-NB // (P * TC)) * (P * TC)
    fb_pad = np.zeros((NB_pad, F), np.float32)
    fb_pad[:NB] = fb

    a1 = a_vec[:F]
    a2 = a_vec[F:]
    Wa2 = (W @ a2).astype(np.float32)
    ba2 = float(b @ a2)

    H = NB_pad // 2
    hflag = (dst >= H).astype(np.int64)
    order = np.lexsort((hflag, src))
    ssrc = src[order]
    sdst = dst[order].astype(np.int64)
    shf = hflag[order]
    deg = np.bincount(ssrc, minlength=NA).astype(np.int64)
    degA = np.bincount(ssrc[shf == 0], minlength=NA).astype(np.int64)
    degB = deg - degA
    row_ptr = np.zeros(NA + 1, np.int64)
    np.cumsum(deg, out=row_ptr[1:])

    nodes_per_core = -(-NA // (ncores * P)) * P          # 6272
    Gc = nodes_per_core // P                             # 49
    nvA = -(-degA // d0)
    nvB = -(-degB // d0)
    KV = max(2, int((nvA + nvB).max()))
    B = 4

    def build_half(lo, hi, degH, nvH, edge_off):
        """Virtual nodes for one dst-half of one core's node range.
        edge_off[n] = first sorted-edge position of this half's run."""
        n_nodes = max(hi - lo, 0)
        node_ids = np.arange(lo, hi)
        nvc = nvH[lo:hi] if n_nodes else np.zeros(0, np.int64)
        Nv = int(nvc.sum())
        vnode = np.repeat(node_ids, nvc)
        vstart0 = np.concatenate([[0], np.cumsum(nvc)])[:-1]
        vrank = np.arange(Nv) - np.repeat(vstart0, nvc)
        pos = edge_off[vnode][:, None] + vrank[:, None] * d0 + np.arange(d0)[None, :]
        valid = (vrank[:, None] * d0 + np.arange(d0)[None, :]) < degH[vnode][:, None]
        posc = np.clip(pos, 0, max(len(sdst) - 1, 0))
        sidx = np.where(valid, sdst[posc] if len(sdst) else 0, 0).astype(np.int64)
        return dict(Nv=Nv, vnode=vnode, nvc=nvc, vstart0=vstart0,
                    sidx=sidx, valid=valid)

    offA = row_ptr[:-1]            # A-run starts at the node's run start
    offB = row_ptr[:-1] + degA     # B-run follows
    cores = []
    for c in range(ncores):
        lo = c * nodes_per_core
        hi = min(lo + nodes_per_core, NA)
        ha = build_half(lo, hi, degA, nvA, offA)
        hb = build_half(lo, hi, degB, nvB, offB)
        hb["sidx"] = np.where(hb["valid"], hb["sidx"] - H, 0)
        cores.append((ha, hb))

    def cdiv(a, b):
        return -(-a // b)

    maxA = max(1, max(h[0]["Nv"] for h in cores))
    maxB = max(h[1]["Nv"] for h in cores)
    GvA = cdiv(cdiv(maxA, P), B) * B
    GvB = cdiv(cdiv(maxB, P), B) * B if maxB > 0 else 0
    NIa = GvA // B
    Gv = GvA + GvB
    Nvp = Gv * P
    B2 = min(4, max(1, 1024 // (KV * P)))
    Gc2 = cdiv(Gc, B2) * B2

    in_maps = []
    PKW = F + d0
    S = B * d0
    NI = Gv // B
    for c in range(ncores):
        ha, hb = cores[c]
        pk = np.zeros((Nvp, PKW), np.float32)
        sidx_all = np.zeros((Nvp, d0), np.int64)
        for (h, base) in ((ha, 0), (hb, GvA * P)):
            Nv = h["Nv"]
            if Nv:
                pk[base:base + Nv, 0:F] = fa[h["vnode"]]
                pk[base:base + Nv, F:F + d0] = h["valid"].astype(np.float32)
                sidx_all[base:base + Nv] = h["sidx"]
        pk = pk.reshape(Gv, P, PKW)

        # int16 gather indices: per batch, flat[(b*d0+k)*128 + p] =
        # sidx[group g0+b, partition p, slot k]; sbuf wrap [16, S*128//16],
        # replicated to 128 partitions; stored bitcast-f32 inside pk so one
        # DMA loads fa_v + mask + idx.  Device reads pk3[:, :, F+D0:] as the
        # per-iteration [P, S*P//16] i16 block, so per-group cols must hold
        # that group's quarter of the batch block: columns [b*d0*8*(..)].
        sidx_g = sidx_all.reshape(Gv, P, d0)
        gidx16 = np.zeros((NI, P, S * P // 16), np.int16)
        for i in range(NI):
            blk = sidx_g[i * B:(i + 1) * B]              # [B, P, d0]
            flat = blk.transpose(0, 2, 1).reshape(-1)    # [(b k) p]
            sb = flat.reshape(S * P // 16, 16).T.astype(np.int16)
            gidx16[i] = np.tile(sb, (8, 1))
        assert sidx_all.max() < 32768

        cpka = np.zeros((Gc2 * P, KV), np.float32)
        cidxa = np.zeros((Gc2 * P, KV), np.int64)
        n_nodes = min(nodes_per_core, NA - c * nodes_per_core)
        if n_nodes > 0:
            nv_tot = ha["nvc"] + hb["nvc"]
            ks = np.arange(KV)[None, :]
            cvalid = ks < nv_tot[:, None]
            # first the node's A-virtual rows, then its B-virtual rows
            inA = ks < ha["nvc"][:, None]
            idxA = ha["vstart0"][:, None] + ks
            idxB = GvA * P + hb["vstart0"][:, None] + (ks - ha["nvc"][:, None])
            cidxv = np.where(cvalid, np.where(inA, idxA, idxB), 0)
            cpka[:n_nodes, 0:KV] = cvalid.astype(np.float32)
            cidxa[:n_nodes] = cidxv
        cpk = cpka.reshape(Gc2, P, KV)
        assert cidxa.max() < 32768
        # int16 wrap for phase-2 dma_gather, one batch of B2 groups per call:
        # flat[(b*KV + k)*128 + p] = cidx[group r0+b, p, k]
        cg = cidxa.reshape(Gc2, P, KV)
        NW = B2 * KV * P // 16
        cidx16 = np.zeros((Gc2 // B2, P, NW), np.int16)
        for r in range(Gc2 // B2):
            flat = cg[r * B2:(r + 1) * B2].transpose(0, 2, 1).reshape(-1)
            sb = flat.reshape(NW, 16).T.astype(np.int16)
            cidx16[r] = np.tile(sb, (8, 1))

        wvec = np.zeros((P, 3 * F), np.float32)
        wvec[:, 0:F] = a1[None, :]
        wvec[:, F:2 * F] = Wa2[None, :]
        wvec[:, 2 * F:3 * F] = b[None, :]

        in_maps.append(dict(
            fb_tab=fb_pad,
            pk=np.ascontiguousarray(pk),
            gidx=np.ascontiguousarray(gidx16),
            cpk=np.ascontiguousarray(cpk),
            cidx=np.ascontiguousarray(cidx16),
            wvec=wvec,
            wmat=np.ascontiguousarray(W),
        ))

    cfg = dict(Gv=Gv, Gc=Gc, Gc2=Gc2, B2=B2, KV=KV, B=B, ba2=ba2, NB=NB,
               NB_pad=NB_pad, NIa=NIa, H=H, Nvp=Nvp,
               nodes_per_core=nodes_per_core, NA=NA)
    return in_maps, cfg


def build_bass(cfg, ncores=NCORES):
    nc = bacc.Bacc("TRN2", target_bir_lowering=False, debug=False,
                   enable_asserts=False, num_devices=ncores)
    ins = dict(
        fb_tab=nc.dram_tensor("fb_tab", [cfg["NB_pad"], F], f32,
                              kind="ExternalInput").ap(),
        pk=nc.dram_tensor("pk", [cfg["Gv"], P, F + D0], f32,
                          kind="ExternalInput").ap(),
        gidx=nc.dram_tensor("gidx", [cfg["Gv"] // cfg["B"], P,
                                     cfg["B"] * D0 * P // 16], i16,
                            kind="ExternalInput").ap(),
        cpk=nc.dram_tensor("cpk", [cfg["Gc2"], P, cfg["KV"]], f32,
                           kind="ExternalInput").ap(),
        cidx=nc.dram_tensor("cidx", [cfg["Gc2"] // cfg["B2"], P,
                                     cfg["B2"] * cfg["KV"] * P // 16], i16,
                            kind="ExternalInput").ap(),
        wvec=nc.dram_tensor("wvec", [P, 3 * F], f32, kind="ExternalInput").ap(),
        wmat=nc.dram_tensor("wmat", [F, F], f32, kind="ExternalInput").ap(),
    )
    scratch_kind = cfg.get("scratch_kind", "Internal")
    outs = dict(
        out=nc.dram_tensor("out", [cfg["Gc2"] * P, F], f32,
                           kind="ExternalOutput").ap(),
        vtab=nc.dram_tensor("vtab", [cfg["Nvp"], 128], f32,
                            kind=scratch_kind).ap(),
        tab2=nc.dram_tensor("tab2", [cfg["NB_pad"], TW], bf16,
                            kind=scratch_kind).ap(),
    )
    with tile.TileContext(nc) as tc:
        emit_program(tc, ins, outs, cfg)
    nc.compile()
    return nc


# ----------------------------------------------------------------------------
# entry point
# ----------------------------------------------------------------------------

def kernel_with_results(trace=False, **inputs):
    from concourse import bass_utils

    in_maps, cfg = prep_inputs(**inputs)
    nc = build_bass(cfg)
    res = bass_utils.run_bass_kernel_spmd(
        nc, in_maps, core_ids=list(range(NCORES)), trace=trace,
    )
    outs = [r["out"][:cfg["nodes_per_core"]] for r in res.results]
    full = np.concatenate(outs, axis=0)[:cfg["NA"]]
    return full.astype(np.float32), res


def kernel(**inputs):
    return kernel_with_results(trace=False, **inputs)[0]


def kernel_timed(nreps=6, **inputs):
    """Like kernel(), but reuses the compiled PJRT executable and times warm
    repeat executions with device-resident inputs.  Returns (out, [ns,...])."""
    import time
    import jax
    from jax.sharding import Mesh, PartitionSpec, NamedSharding
    from jax.experimental.shard_map import shard_map
    from concourse import bass2jax

    in_maps, cfg = prep_inputs(**inputs)
    nc = build_bass(cfg)
    bass2jax.install_neuronx_cc_hook()

    ncores = NCORES
    partition_name = nc.partition_id_tensor.name if nc.partition_id_tensor else None
    in_names, out_names, out_avals, zero_outs = [], [], [], []
    for alloc in nc.m.functions[0].allocations:
        if not isinstance(alloc, mybir.MemoryLocationSet):
            continue
        name = alloc.memorylocations[0].name
        if alloc.kind == "ExternalInput":
            if name != partition_name:
                in_names.append(name)
        elif alloc.kind == "ExternalOutput":
            shape = tuple(alloc.tensor_shape)
            dtype = mybir.dt.np(alloc.dtype)
            out_avals.append(jax.core.ShapedArray(shape, dtype))
            out_names.append(name)
            zero_outs.append(np.zeros(shape, dtype))
    n_params = len(in_names)
    n_outs = len(out_avals)
    all_in_names = list(in_names) + list(out_names)
    if partition_name is not None:
        all_in_names.append(partition_name)

    def _body(*args):
        operands = list(args)
        if partition_name is not None:
            operands.append(bass2jax.partition_id_tensor())
        outs = bass2jax._bass_exec_p.bind(
            *operands,
            out_avals=tuple(out_avals),
            in_names=tuple(all_in_names),
            out_names=tuple(out_names),
            lowering_input_output_aliases=(),
            sim_require_finite=True,
            sim_require_nnan=True,
            nc=nc,
        )
        return tuple(outs)

    devices = jax.devices()[:ncores]
    mesh = Mesh(np.asarray(devices), ("core",))
    spec = PartitionSpec("core")
    shard = NamedSharding(mesh, spec)
    donate = tuple(range(n_params, n_params + n_outs))
    sharded = jax.jit(
        shard_map(_body, mesh=mesh, in_specs=(spec,) * (n_params + n_outs),
                  out_specs=(spec,) * n_outs, check_rep=False),
        donate_argnums=donate, keep_unused=True,
    )
    concat_in = [
        np.concatenate([np.asarray(in_maps[c][nm]) for c in range(ncores)], axis=0)
        for nm in in_names
    ]
    concat_zeros = [
        np.zeros((ncores * z.shape[0], *z.shape[1:]), z.dtype) for z in zero_outs
    ]
    dev_in = [jax.device_put(a, shard) for a in concat_in]

    out_arrs = None
    times = []
    for rep in range(nreps + 1):
        dz = [jax.device_put(z, shard) for z in concat_zeros]
        for d in dz:
            d.block_until_ready()
        t0 = time.perf_counter()
        res = sharded(*dev_in, *dz)
        for r in res:
            r.block_until_ready()
        t1 = time.perf_counter()
        if rep > 0:
            times.append(int((t1 - t0) * 1e9))
        out_arrs = res

    outs = {}
    for i, name in enumerate(out_names):
        outs[name] = np.asarray(out_arrs[i]).reshape(
            ncores, *out_avals[i].shape)
    full = np.concatenate(
        [outs["out"][c][:cfg["nodes_per_core"]] for c in range(ncores)],
        axis=0)[:cfg["NA"]]
    return full.astype(np.float32), times


if __name__ == "__main__":
    np.random.seed(0)
    NA = NB = 50000
    E = 800000
    ins = dict(
        feature_a=np.random.randn(NA, F).astype(np.float32),
        feature_b=np.random.randn(NB, F).astype(np.float32),
        W=(np.random.randn(F, F) / 8).astype(np.float32),
        b=np.zeros(F, np.float32),
        a_vec=(np.random.randn(2 * F, 1) * 0.05).astype(np.float32),
        edges=np.stack([np.random.randint(0, NA, E),
                        np.random.randint(0, NB, E)], 1).astype(np.int64),
        node_num_a=NA,
    )
    out = kernel(**ins)
    print(out.shape, out.dtype)



# revision 14
# speedup vs baseline: 3.3195x; 1.1527x over previous
"""Trainium2 Bass kernel for AttentionAggregator (GNN message passing).

Reference computation:
    new_emb = fb @ W + b
    s_e     = (fa @ a1)[src_e] + (new_emb @ a2)[dst_e]
    score_e = exp(elu(s_e, 0.1))
    out[n]  = (sum_{e: src_e=n} score_e * new_emb[dst_e]) / max(den[n], 1 if 0)

Algebraic reformulation (linearity of the segment sum):
    q_e   = fb[dst_e] @ (W @ a2)            # per-edge scalar
    s_e   = (fa @ a1)[src_e] + q_e + b @ a2
    G[n]  = sum_e score_e * fb[dst_e]       # [Na, 64]
    den[n]= sum_e score_e
    out[n]= (G[n] @ W) / den_safe[n] + 1[den[n] > 0] * b

(the scalar divide commutes with @W, so no new_emb and no pre-divide.)

Distribution: nodes sharded contiguously across 8 cores (6250 each); edges
sorted by src on host, so each core owns its nodes' full edge lists.  fb is
replicated; no collective needed.

Device algorithm (single pass, no scratch):
  Nodes of a core are sorted by degree and processed 128 per iteration, one
  node per partition, D_it slot columns (D_it = padded max degree of the
  batch across all cores, so one program serves all cores).  Each slot
  fetches fb[dst] directly from the replicated f32 fb table with a 512-byte
  dma_gather of the node PAIR (idx = dst>>1 keeps indices int16); a
  host-provided parity plane selects the correct half by weighting
  (score*par / score*(1-par)) at accumulation time.  q_e is computed on the
  fly from the gathered rows, so no augmented table is ever built.  Gathers
  are spread over 4 SWDGE queues (4x descriptor-generation parallelism).
  Per-batch: scores on ACT/DVE, weighted reduce to G[128,64], PE transpose +
  matmul for G@W, per-partition divide by den, +b, sequential out DMA.
  Iterations with equal D are emitted as one op group to amortize
  instruction overheads.
"""

import sys

for _p in ("/opt/trn_rl_repo",):
    if _p not in sys.path:
        sys.path.insert(0, _p)

import numpy as np

import concourse.bass as bass
import concourse.bacc as bacc
import concourse.mybir as mybir
import concourse.tile as tile
from concourse.masks import make_identity

P = 128
F = 64          # feature dim
NCORES = 8
NA = 50000
NB = 50000
NPC = NA // NCORES              # nodes per core (6250)
NIT = -(-NPC // P)              # iterations (49)
NROWS = NIT * P                 # padded nodes per core (6272)

f32 = mybir.dt.float32
bf16 = mybir.dt.bfloat16
i16 = mybir.dt.int16
AX = mybir.AxisListType
OP = mybir.AluOpType
ACTF = mybir.ActivationFunctionType
MAX_IDX_PER_CALL = 1024         # SWDGE descriptor-ring capacity
NQ = 4                          # SWDGE queues


# ----------------------------------------------------------------------------
# device program
# ----------------------------------------------------------------------------

def emit_program(tc, ins, outs, cfg):
    nc = tc.nc
    groups = cfg["groups"]        # list of (D, B) — B iterations of width D
    ba2 = float(cfg["ba2"])
    MDW = cfg["MDW"]              # pk_md total width (sum of 2*D*B)
    GW = cfg["GW"]                # gidx total width (sum of S/16 per iter)
    fb_tab = ins["fb_tab"]        # [NB//2, 2*F] f32 (512B node-pair rows)
    pk_fa = ins["pk_fa"]          # [P, NIT*F]
    pk_md = ins["pk_md"]          # [P, MDW]  per-iter [par(D) | npar(D)]
    gidx = ins["gidx"]            # [P, GW] i16
    wvec = ins["wvec"]            # [P, 3*F]  a1 | Wa2 | b
    wmat = ins["wmat"]            # [F, F]
    out = outs["out"]             # [NROWS, F] iteration-ordered

    with (
        tc.tile_pool(name="const", bufs=1) as cpool,
        tc.tile_pool(name="work", bufs=3) as pool,
        tc.tile_pool(name="big", bufs=2) as bigpool,
        tc.tile_pool(name="psum", bufs=4, space="PSUM") as psum,
    ):
        wvec_t = cpool.tile([P, 3 * F], f32)
        nc.sync.dma_start(out=wvec_t[:], in_=wvec)
        wmat_t = cpool.tile([P, F], f32)
        nc.sync.dma_start(out=wmat_t[0:F, :], in_=wmat)
        nc.sync.dma_start(out=wmat_t[F:2 * F, :], in_=wmat)
        ident = cpool.tile([P, P], f32)
        make_identity(nc, ident[:])
        zbias = cpool.tile([P, 1], f32)
        nc.vector.memset(zbias[:], 0.0)
        mbias = cpool.tile([P, 1], f32)
        nc.vector.memset(mbias[:], -0.1)
        a1v = wvec_t[:, 0:F]
        w2v = wvec_t[:, F:2 * F]
        bv = wvec_t[:, 2 * F:3 * F]

        fa_t = cpool.tile([P, NIT * F], f32)
        nc.sync.dma_start(out=fa_t[:], in_=pk_fa)
        md_t = cpool.tile([P, MDW], f32)
        nc.sync.dma_start(out=md_t[:], in_=pk_md)
        gi_t = cpool.tile([P, GW], i16)
        nc.sync.dma_start(out=gi_t[:], in_=gidx)

        # e1[p, it] = fa[p, it, :] @ a1 + ba2, for all iterations at once
        faprod = cpool.tile([P, NIT * F], f32)
        nc.vector.tensor_tensor(
            out=faprod[:].rearrange("p (i f) -> p i f", f=F),
            in0=fa_t[:].rearrange("p (i f) -> p i f", f=F),
            in1=a1v[:, None, :].to_broadcast([P, NIT, F]),
            op=OP.mult,
        )
        e1 = cpool.tile([P, NIT], f32)
        nc.vector.tensor_reduce(
            out=e1[:],
            in_=faprod[:].rearrange("p (i f) -> p i f", f=F),
            axis=AX.X, op=OP.add,
        )
        if ba2 != 0.0:
            nc.vector.tensor_scalar(
                out=e1[:], in0=e1[:], scalar1=ba2, scalar2=None, op0=OP.add,
            )

        fb2 = fb_tab                      # [NB//2, 128] f32, 512B rows
        out3 = out.rearrange("(i p) f -> i p f", p=P)

        it0 = 0            # first iteration of this group
        md_off = 0         # running col offset into md_t
        gi_off = 0         # running col offset into gi_t (int16 cols)
        call_i = [0]       # global gather-call counter for queue rotation

        for rep in range(cfg.get("rep1", 1)):
            it0 = 0
            md_off = 0
            gi_off = 0
            for (D, B) in groups:
                S = P * D                  # slots (= gather idxs) per iter
                DB = D * B
                # ---- gather: B iterations' rows --------------------------
                rows = bigpool.tile([P, DB * 2 * F], f32, tag="rows")
                rows4 = rows[:].rearrange("p (m w) -> p m w", w=2 * F)
                for b in range(B):
                    off = 0
                    while off < S:
                        n = min(MAX_IDX_PER_CALL, S - off)
                        o0 = b * D + off // P
                        nc.gpsimd.dma_gather(
                            out_ap=rows4[:, o0:o0 + n // P, :],
                            in_ap=fb2,
                            idxs_ap=gi_t[:, gi_off + (b * S + off) // 16:
                                         gi_off + (b * S + off + n) // 16],
                            num_idxs=n,
                            num_idxs_reg=n,
                            elem_size=2 * F,
                            queue_num=call_i[0] % NQ,
                        )
                        call_i[0] += 1
                        off += n
                # ---- q per slot ------------------------------------------
                qprod = bigpool.tile([P, DB * 2 * F], bf16, tag="qprod")
                nc.vector.tensor_tensor(
                    out=qprod[:].rearrange("p (m f) -> p m f", f=F),
                    in0=rows[:].rearrange("p (m f) -> p m f", f=F),
                    in1=w2v[:, None, :].to_broadcast([P, DB * 2, F]),
                    op=OP.mult,
                )
                q2 = pool.tile([P, DB * 2], f32, tag="q2")
                nc.vector.tensor_reduce(
                    out=q2[:],
                    in_=qprod[:].rearrange("p (m f) -> p m f", f=F),
                    axis=AX.X, op=OP.add,
                )
                # parity blend: q = q_even + (q_odd - q_even) * par
                q23 = q2[:].rearrange("p (m two) -> p m two", two=2)
                par = md_t[:, md_off:md_off + DB]
                npar = md_t[:, md_off + DB:md_off + 2 * DB]
                qd = pool.tile([P, DB], f32, tag="qd")
                nc.vector.tensor_tensor(
                    out=qd[:], in0=q23[:, :, 1], in1=q23[:, :, 0], op=OP.subtract,
                )
                nc.vector.tensor_tensor(
                    out=qd[:], in0=qd[:], in1=par, op=OP.mult,
                )
                s_t = pool.tile([P, DB], f32, tag="s")
                nc.vector.tensor_tensor(
                    out=s_t[:], in0=qd[:], in1=q23[:, :, 0], op=OP.add,
                )
                nc.vector.tensor_tensor(
                    out=s_t[:].rearrange("p (b d) -> p b d", d=D),
                    in0=s_t[:].rearrange("p (b d) -> p b d", d=D),
                    in1=e1[:, it0:it0 + B, None].to_broadcast([P, B, D]),
                    op=OP.add,
                )
                # score = where(s > 0, exp(s), exp(0.1*exp(s) - 0.1))
                t_t = pool.tile([P, DB], f32, tag="t")
                nc.scalar.activation(t_t[:], s_t[:], ACTF.Exp,
                                     bias=zbias[:, 0:1], scale=1.0)
                u_t = pool.tile([P, DB], f32, tag="u")
                nc.scalar.activation(u_t[:], t_t[:], ACTF.Exp,
                                     bias=mbias[:, 0:1], scale=0.1)
                m_t = pool.tile([P, DB], mybir.dt.uint8, tag="m")
                nc.vector.tensor_scalar(
                    out=m_t[:], in0=s_t[:], scalar1=0.0, scalar2=None,
                    op0=OP.is_gt,
                )
                nc.vector.copy_predicated(out=u_t[:], mask=m_t[:], data=t_t[:])
                # per-parity weights (padding slots have par = npar = 0)
                w2t = pool.tile([P, DB * 2], f32, tag="w2")
                w23 = w2t[:].rearrange("p (m two) -> p m two", two=2)
                nc.vector.tensor_tensor(
                    out=w23[:, :, 0], in0=u_t[:], in1=npar, op=OP.mult,
                )
                nc.vector.tensor_tensor(
                    out=w23[:, :, 1], in0=u_t[:], in1=par, op=OP.mult,
                )
                # weighted rows and reduce to G, den
                scaled = bigpool.tile([P, DB * 2 * F], bf16, tag="scaled")
                nc.vector.tensor_tensor(
                    out=scaled[:].rearrange("p (m f) -> p m f", f=F),
                    in0=rows[:].rearrange("p (m f) -> p m f", f=F),
                    in1=w2t[:, :, None].to_broadcast([P, DB * 2, F]),
                    op=OP.mult,
                )
                g_t = pool.tile([P, B * F], f32, tag="g")
                nc.vector.tensor_reduce(
                    out=g_t[:].rearrange("p (b f) -> p b f", f=F),
                    in_=scaled[:].rearrange("p (b m f) -> p b f m", m=2 * D, f=F),
                    axis=AX.X, op=OP.add,
                )
                den = pool.tile([P, B], f32, tag="den")
                nc.vector.tensor_reduce(
                    out=den[:],
                    in_=w2t[:].rearrange("p (b m) -> p b m", m=2 * D),
                    axis=AX.X, op=OP.add,
                )
                # divide-after-matmul epilogue
                m0 = pool.tile([P, B], f32, tag="m0")
                nc.vector.tensor_scalar(
                    out=m0[:], in0=den[:], scalar1=0.0, scalar2=None,
                    op0=OP.is_equal,
                )
                dsafe = pool.tile([P, B], f32, tag="dsafe")
                nc.vector.tensor_tensor(
                    out=dsafe[:], in0=den[:], in1=m0[:], op=OP.add,
                )
                rec = pool.tile([P, B], f32, tag="rec")
                nc.vector.reciprocal(rec[:], dsafe[:])
                w1 = pool.tile([P, B], f32, tag="w1")
                nc.vector.tensor_scalar(
                    out=w1[:], in0=den[:], scalar1=0.0, scalar2=None,
                    op0=OP.is_gt,
                )
                outs_t = pool.tile([P, B * F], f32, tag="outs")
                for b0 in range(0, B, 2):
                    bw = min(2, B - b0)
                    gtp = psum.tile([bw * F, P], f32, tag="gtp")
                    nc.tensor.transpose(
                        out=gtp[:], in_=g_t[:, b0 * F:(b0 + bw) * F],
                        identity=ident[:])
                    gts = pool.tile([bw * F, P], f32, tag="gts")
                    nc.vector.tensor_copy(out=gts[:], in_=gtp[:])
                    for bb in range(bw):
                        b = b0 + bb
                        h_p = psum.tile([P, F], f32, tag="hp")
                        nc.tensor.matmul(
                            out=h_p[:],
                            lhsT=gts[bb * F:(bb + 1) * F, :],
                            rhs=wmat_t[bb * F:(bb + 1) * F, :],
                            start=True, stop=True)
                        hd = pool.tile([P, F], f32, tag="hd")
                        nc.vector.tensor_scalar(
                            out=hd[:], in0=h_p[:], scalar1=rec[:, b:b + 1],
                            scalar2=None, op0=OP.mult,
                        )
                        badd = pool.tile([P, F], f32, tag="badd")
                        nc.vector.tensor_scalar(
                            out=badd[:], in0=bv, scalar1=w1[:, b:b + 1],
                            scalar2=None, op0=OP.mult,
                        )
                        nc.vector.tensor_tensor(
                            out=outs_t[:, b * F:(b + 1) * F], in0=hd[:],
                            in1=badd[:], op=OP.add,
                        )
                nc.sync.dma_start(
                    out=out3[it0:it0 + B].rearrange("i p f -> p i f"),
                    in_=outs_t[:].rearrange("p (b f) -> p b f", f=F),
                )
                it0 += B
                md_off += 2 * DB
                gi_off += B * S // 16


# ----------------------------------------------------------------------------
# host-side preparation (index plumbing only; host math is W @ a2 / b @ a2)
# ----------------------------------------------------------------------------

def prep_inputs(feature_a, feature_b, W, b, a_vec, edges, node_num_a,
                ncores=NCORES):
    fa = np.asarray(feature_a, np.float32)
    fb = np.asarray(feature_b, np.float32)
    W = np.asarray(W, np.float32)
    b = np.asarray(b, np.float32)
    a_vec = np.asarray(a_vec, np.float32).reshape(-1)
    edges = np.asarray(edges)
    na = int(node_num_a)
    assert na == NA and fb.shape == (NB, F) and fa.shape[1] == F

    a1 = a_vec[:F]
    a2 = a_vec[F:]
    Wa2 = (W @ a2).astype(np.float32)
    ba2 = float(b @ a2)

    src = edges[:, 0].astype(np.int64)
    dst = edges[:, 1].astype(np.int64)
    order = np.argsort(src, kind="stable")
    ssrc = src[order]
    sdst = dst[order]
    deg = np.bincount(ssrc, minlength=na).astype(np.int64)
    row_ptr = np.zeros(na + 1, np.int64)
    np.cumsum(deg, out=row_ptr[1:])

    # per-core degree-sorted batches of 128 nodes
    perms = []          # per core: node id per out row (or -1 for padding)
    Dmat = np.zeros((ncores, NIT), np.int64)
    for c in range(ncores):
        lo = c * NPC
        nodes = np.arange(lo, lo + NPC)
        p = nodes[np.argsort(deg[nodes], kind="stable")]
        p = np.concatenate([np.full(NROWS - NPC, -1, np.int64), p])
        # padding rows first (degree 0), keeps batches degree-sorted
        perms.append(p)
        dpad = np.concatenate([np.zeros(NROWS - NPC, np.int64), deg[p[NROWS - NPC:]]])
        Dmat[c] = dpad.reshape(NIT, P).max(axis=1)
    D_it = np.maximum(Dmat.max(axis=0), 1)       # shared widths across cores

    # group consecutive iterations of equal D (bounded group size)
    groups = []
    i = 0
    while i < NIT:
        j = i
        while j < NIT and D_it[j] == D_it[i] and (j - i) < 8 \
                and (j - i + 1) * D_it[i] <= 48:
            j += 1
        groups.append((int(D_it[i]), j - i))
        i = j

    MDW = int(sum(2 * D * B for D, B in groups))
    GW = int(sum(P * D * B // 16 for D, B in groups))

    in_maps = []
    for c in range(ncores):
        p = perms[c]
        pk_fa = np.zeros((P, NIT * F), np.float32)
        pk_md = np.zeros((P, MDW), np.float32)
        gidx = np.zeros((P, GW), np.int16)
        md_off = 0
        gi_off = 0
        it0 = 0
        for (D, B) in groups:
            S = P * D
            for bi in range(B):
                it = it0 + bi
                nid = p[it * P:(it + 1) * P]                 # [P]
                valid_n = nid >= 0
                nid_c = np.where(valid_n, nid, 0)
                pk_fa[:, it * F:(it + 1) * F] = np.where(
                    valid_n[:, None], fa[nid_c], 0.0)
                dg = np.where(valid_n, deg[nid_c], 0)         # [P]
                ks = np.arange(D)[None, :]                    # [1, D]
                vmask = ks < dg[:, None]                      # [P, D]
                pos = row_ptr[nid_c][:, None] + ks
                pos = np.clip(pos, 0, len(sdst) - 1)
                d_all = np.where(vmask, sdst[pos], 0)         # [P, D]
                par = (d_all & 1) & vmask
                npar = vmask & ~(d_all & 1).astype(bool)
                # group layout: par for all B iters [DB], then npar [DB]
                DB = D * B
                o = md_off + D * bi
                pk_md[:, o:o + D] = par.astype(np.float32)
                pk_md[:, DB + o:DB + o + D] = npar.astype(np.float32)
                idx = (d_all >> 1).astype(np.int16)           # [P, D]
                flat = idx.T.reshape(-1)                      # [(k p)] p-fastest
                sb = flat.reshape(S // 16, 16).T              # [16, S/16]
                go = gi_off + bi * S // 16
                gidx[:, go:go + S // 16] = np.tile(sb, (8, 1))
            it0 += B
            md_off += 2 * D * B
            gi_off += B * S // 16
        assert gidx.max() < 32768 and (NB - 1) >> 1 < 32768

        wvec = np.zeros((P, 3 * F), np.float32)
        wvec[:, 0:F] = a1[None, :]
        wvec[:, F:2 * F] = Wa2[None, :]
        wvec[:, 2 * F:3 * F] = b[None, :]
        in_maps.append(dict(
            fb_tab=np.ascontiguousarray(fb.reshape(NB // 2, 2 * F)),
            pk_fa=pk_fa,
            pk_md=pk_md,
            gidx=gidx,
            wvec=wvec,
            wmat=np.ascontiguousarray(W),
        ))

    cfg = dict(groups=groups, MDW=MDW, GW=GW, ba2=ba2, perms=perms)
    return in_maps, cfg


def build_bass(cfg, ncores=NCORES):
    nc = bacc.Bacc("TRN2", target_bir_lowering=False, debug=False,
                   enable_asserts=False, num_devices=ncores,
                   num_swdge_queues=NQ)
    ins = dict(
        fb_tab=nc.dram_tensor("fb_tab", [NB // 2, 2 * F], f32,
                              kind="ExternalInput").ap(),
        pk_fa=nc.dram_tensor("pk_fa", [P, NIT * F], f32,
                             kind="ExternalInput").ap(),
        pk_md=nc.dram_tensor("pk_md", [P, cfg["MDW"]], f32,
                             kind="ExternalInput").ap(),
        gidx=nc.dram_tensor("gidx", [P, cfg["GW"]], i16,
                            kind="ExternalInput").ap(),
        wvec=nc.dram_tensor("wvec", [P, 3 * F], f32, kind="ExternalInput").ap(),
        wmat=nc.dram_tensor("wmat", [F, F], f32, kind="ExternalInput").ap(),
    )
    outs = dict(
        out=nc.dram_tensor("out", [NROWS, F], f32, kind="ExternalOutput").ap(),
    )
    with tile.TileContext(nc) as tc:
        emit_program(tc, ins, outs, cfg)
    nc.compile()
    return nc


def assemble_output(results, cfg):
    full = np.zeros((NA, F), np.float32)
    for c in range(NCORES):
        p = cfg["perms"][c]
        rows = results[c]["out"]
        valid = p >= 0
        full[p[valid]] = rows[valid]
    return full


# ----------------------------------------------------------------------------
# entry points
# ----------------------------------------------------------------------------

def kernel_with_results(trace=False, **inputs):
    from concourse import bass_utils

    in_maps, cfg = prep_inputs(**inputs)
    nc = build_bass(cfg)
    res = bass_utils.run_bass_kernel_spmd(
        nc, in_maps, core_ids=list(range(NCORES)), trace=trace,
    )
    return assemble_output(res.results, cfg), res


def kernel(**inputs):
    return kernel_with_results(trace=False, **inputs)[0]


def kernel_timed(nreps=6, rep1=1, **inputs):
    """Reuses the compiled PJRT executable; times warm repeat executions with
    device-resident inputs.  Returns (out, [ns,...])."""
    import time
    import jax
    from jax.sharding import Mesh, PartitionSpec, NamedSharding
    from jax.experimental.shard_map import shard_map
    from concourse import bass2jax

    in_maps, cfg = prep_inputs(**inputs)
    cfg["rep1"] = rep1
    nc = build_bass(cfg)
    bass2jax.install_neuronx_cc_hook()

    ncores = NCORES
    partition_name = nc.partition_id_tensor.name if nc.partition_id_tensor else None
    in_names, out_names, out_avals, zero_outs = [], [], [], []
    for alloc in nc.m.functions[0].allocations:
        if not isinstance(alloc, mybir.MemoryLocationSet):
            continue
        name = alloc.memorylocations[0].name
        if alloc.kind == "ExternalInput":
            if name != partition_name:
                in_names.append(name)
        elif alloc.kind == "ExternalOutput":
            shape = tuple(alloc.tensor_shape)
            dtype = mybir.dt.np(alloc.dtype)
            out_avals.append(jax.core.ShapedArray(shape, dtype))
            out_names.append(name)
            zero_outs.append(np.zeros(shape, dtype))
    n_params = len(in_names)
    n_outs = len(out_avals)
    all_in_names = list(in_names) + list(out_names)
    if partition_name is not None:
        all_in_names.append(partition_name)

    def _body(*args):
        operands = list(args)
        if partition_name is not None:
            operands.append(bass2jax.partition_id_tensor())
        outs_ = bass2jax._bass_exec_p.bind(
            *operands,
            out_avals=tuple(out_avals),
            in_names=tuple(all_in_names),
            out_names=tuple(out_names),
            lowering_input_output_aliases=(),
            sim_require_finite=True,
            sim_require_nnan=True,
            nc=nc,
        )
        return tuple(outs_)

    devices = jax.devices()[:ncores]
    mesh = Mesh(np.asarray(devices), ("core",))
    spec = PartitionSpec("core")
    shard = NamedSharding(mesh, spec)
    sharded = jax.jit(
        shard_map(_body, mesh=mesh, in_specs=(spec,) * (n_params + n_outs),
                  out_specs=(spec,) * n_outs, check_rep=False),
        keep_unused=True,
    )
    concat_in = [
        np.concatenate([np.asarray(in_maps[c][nm]) for c in range(ncores)],
                       axis=0)
        for nm in in_names
    ]
    concat_zeros = [
        np.zeros((ncores * z.shape[0], *z.shape[1:]), z.dtype) for z in zero_outs
    ]
    dev_in = [jax.device_put(a, shard) for a in concat_in]
    dzs = [jax.device_put(z, shard) for z in concat_zeros]

    out_arrs = None
    times = []
    for rep in range(nreps + 1):
        t0 = time.perf_counter()
        res = sharded(*dev_in, *dzs)
        for r in res:
            r.block_until_ready()
        t1 = time.perf_counter()
        if rep > 0:
            times.append(int((t1 - t0) * 1e9))
        out_arrs = res

    results = []
    for c in range(ncores):
        m = {}
        for i, name in enumerate(out_names):
            m[name] = np.asarray(out_arrs[i]).reshape(
                ncores, *out_avals[i].shape)[c]
        results.append(m)
    return assemble_output(results, cfg), times


if __name__ == "__main__":
    np.random.seed(0)
    E = 800000
    ins = dict(
        feature_a=np.random.randn(NA, F).astype(np.float32),
        feature_b=np.random.randn(NB, F).astype(np.float32),
        W=(np.random.randn(F, F) / 8).astype(np.float32),
        b=np.zeros(F, np.float32),
        a_vec=(np.random.randn(2 * F, 1) * 0.05).astype(np.float32),
        edges=np.stack([np.random.randint(0, NA, E),
                        np.random.randint(0, NB, E)], 1).astype(np.int64),
        node_num_a=NA,
    )
    out = kernel(**ins)
    print(out.shape, out.dtype)


# revision 16
# speedup vs baseline: 3.5132x; 1.0584x over previous
"""Trainium2 Bass kernel for AttentionAggregator (GNN message passing).

Reference computation:
    new_emb = fb @ W + b
    s_e     = (fa @ a1)[src_e] + (new_emb @ a2)[dst_e]
    score_e = exp(elu(s_e, 0.1))
    out[n]  = (sum_{e: src_e=n} score_e * new_emb[dst_e]) / max(den[n], 1 if 0)

Algebraic reformulation (linearity of the segment sum):
    q_e   = fb[dst_e] @ (W @ a2)            # per-edge scalar
    s_e   = (fa @ a1)[src_e] + q_e + b @ a2
    G[n]  = sum_e score_e * fb[dst_e]       # [Na, 64]
    den[n]= sum_e score_e
    out[n]= (G[n] @ W) / den_safe[n] + 1[den[n] > 0] * b

(the scalar divide commutes with @W, so no new_emb and no pre-divide.)

Distribution: nodes sharded contiguously across 8 cores (6250 each); edges
sorted by src on host, so each core owns its nodes' full edge lists.  fb is
replicated; no collective needed.

Device algorithm (single pass, no scratch):
  Nodes of a core are sorted by degree and processed 128 per iteration, one
  node per partition, D_it slot columns (D_it = padded max degree of the
  batch across all cores, so one program serves all cores).  Each slot
  fetches fb[dst] directly from the replicated f32 fb table with a 512-byte
  dma_gather of the node PAIR (idx = dst>>1 keeps indices int16); a
  host-provided parity plane selects the correct half by weighting
  (score*par / score*(1-par)) at accumulation time.  q_e is computed on the
  fly from the gathered rows, so no augmented table is ever built.  Gathers
  are spread over 4 SWDGE queues (4x descriptor-generation parallelism).
  Per-batch: scores on ACT/DVE, weighted reduce to G[128,64], PE transpose +
  matmul for G@W, per-partition divide by den, +b, sequential out DMA.
  Iterations with equal D are emitted as one op group to amortize
  instruction overheads.
"""

import sys

for _p in ("/opt/trn_rl_repo",):
    if _p not in sys.path:
        sys.path.insert(0, _p)

import numpy as np

import concourse.bass as bass
import concourse.bacc as bacc
import concourse.mybir as mybir
import concourse.tile as tile
from concourse.masks import make_identity

P = 128
F = 64          # feature dim
NCORES = 8
NA = 50000
NB = 50000
NPC = NA // NCORES              # nodes per core (6250)
NIT = -(-NPC // P)              # iterations (49)
NROWS = NIT * P                 # padded nodes per core (6272)

f32 = mybir.dt.float32
bf16 = mybir.dt.bfloat16
i16 = mybir.dt.int16
AX = mybir.AxisListType
OP = mybir.AluOpType
ACTF = mybir.ActivationFunctionType
MAX_IDX_PER_CALL = 1024         # SWDGE descriptor-ring capacity
NQ = 4                          # SWDGE queues


# ----------------------------------------------------------------------------
# device program
# ----------------------------------------------------------------------------

def emit_program(tc, ins, outs, cfg):
    nc = tc.nc
    groups = cfg["groups"]        # list of (D, B) -- B iterations of width D
    ba2 = float(cfg["ba2"])
    MDW = cfg["MDW"]              # pk_md width: sum of 2*D*B (interleaved mask)
    GW = cfg["GW"]                # gidx width: sum of S/16 per iter
    fb_tab = ins["fb_tab"]        # [NB//2, 2*F] f32 (512B node-pair rows)
    pk_fa = ins["pk_fa"]          # [P, NIT*F]
    pk_md = ins["pk_md"]          # [P, MDW]: per slot [even-valid, odd-valid]
    gidx = ins["gidx"]            # [P, GW] i16
    wvec = ins["wvec"]            # [P, 3*F]  a1 | Wa2 | b
    wmat = ins["wmat"]            # [F, F]
    out = outs["out"]             # [NROWS, F] iteration-ordered

    G1 = F + 1                    # packed row: G (64) | den
    with (
        tc.tile_pool(name="const", bufs=1) as cpool,
        tc.tile_pool(name="work", bufs=3) as pool,
        tc.tile_pool(name="big", bufs=2) as bigpool,
        tc.tile_pool(name="rowsp", bufs=2) as rowspool,
        tc.tile_pool(name="psum", bufs=4, space="PSUM") as psum,
    ):
        wvec_t = cpool.tile([P, 3 * F], f32)
        nc.sync.dma_start(out=wvec_t[:], in_=wvec)
        a1v = wvec_t[:, 0:F]
        w2v = wvec_t[:, F:2 * F]
        bv = wvec_t[:, 2 * F:3 * F]
        # wb65: rows 0..63 = W, row 64 = b   (rhs for the packed matmul)
        wb65 = cpool.tile([G1, F], f32)
        nc.sync.dma_start(out=wb65[0:F, :], in_=wmat)
        nc.sync.dma_start(out=wb65[F:G1, :], in_=wvec[0:1, 2 * F:3 * F])
        ident = cpool.tile([P, P], f32)
        make_identity(nc, ident[:])
        zbias = cpool.tile([P, 1], f32)
        nc.vector.memset(zbias[:], 0.0)
        mbias = cpool.tile([P, 1], f32)
        nc.vector.memset(mbias[:], -0.1)

        fa_t = cpool.tile([P, NIT * F], f32)
        nc.sync.dma_start(out=fa_t[:], in_=pk_fa)
        md_t = cpool.tile([P, MDW], f32)
        nc.sync.dma_start(out=md_t[:], in_=pk_md)
        gi_t = cpool.tile([P, GW], i16)
        nc.sync.dma_start(out=gi_t[:], in_=gidx)
        h_all = cpool.tile([P, NIT * F], f32)
        den_all = cpool.tile([P, NIT], f32)

        # e1[p, it] = fa[p, it, :] @ a1 + ba2, for all iterations at once
        faprod = bigpool.tile([P, NIT * F], f32, tag="outs")
        nc.vector.tensor_tensor(
            out=faprod[:].rearrange("p (i f) -> p i f", f=F),
            in0=fa_t[:].rearrange("p (i f) -> p i f", f=F),
            in1=a1v[:, None, :].to_broadcast([P, NIT, F]),
            op=OP.mult,
        )
        e1 = cpool.tile([P, NIT], f32)
        nc.vector.tensor_reduce(
            out=e1[:],
            in_=faprod[:].rearrange("p (i f) -> p i f", f=F),
            axis=AX.X, op=OP.add,
        )
        if ba2 != 0.0:
            nc.vector.tensor_scalar(
                out=e1[:], in0=e1[:], scalar1=ba2, scalar2=None, op0=OP.add,
            )

        fb2 = fb_tab                      # [NB//2, 128] f32, 512B rows
        out3 = out.rearrange("(i p) f -> i p f", p=P)
        call_i = [0]

        for rep in range(cfg.get("rep1", 1)):
            it0 = 0
            md_off = 0
            gi_off = 0
            for (D, B) in groups:
                S = P * D                  # slots (= gather idxs) per iter
                DB = D * B
                M2 = DB * 2                # slot-parity lanes
                # ---- gather: B iterations' 512B pair rows ----------------
                rows = rowspool.tile([P, M2 * F], f32, tag="rows")
                rows4 = rows[:].rearrange("p (m w) -> p m w", w=2 * F)
                for b in range(B):
                    off = 0
                    while off < S:
                        n = min(MAX_IDX_PER_CALL, S - off)
                        o0 = b * D + off // P
                        nc.gpsimd.dma_gather(
                            out_ap=rows4[:, o0:o0 + n // P, :],
                            in_ap=fb2,
                            idxs_ap=gi_t[:, gi_off + (b * S + off) // 16:
                                         gi_off + (b * S + off + n) // 16],
                            num_idxs=n,
                            num_idxs_reg=n,
                            elem_size=2 * F,
                            queue_num=call_i[0] % NQ,
                        )
                        call_i[0] += 1
                        off += n
                # ---- q for both parities ---------------------------------
                qprod = bigpool.tile([P, M2 * F], bf16, tag="qprod")
                nc.vector.tensor_tensor(
                    out=qprod[:].rearrange("p (m f) -> p m f", f=F),
                    in0=rows[:].rearrange("p (m f) -> p m f", f=F),
                    in1=w2v[:, None, :].to_broadcast([P, M2, F]),
                    op=OP.mult,
                )
                s2 = pool.tile([P, M2], f32, tag="s2")
                nc.vector.tensor_reduce(
                    out=s2[:],
                    in_=qprod[:].rearrange("p (m f) -> p m f", f=F),
                    axis=AX.X, op=OP.add,
                )
                nc.vector.tensor_tensor(
                    out=s2[:].rearrange("p (b m) -> p b m", m=2 * D),
                    in0=s2[:].rearrange("p (b m) -> p b m", m=2 * D),
                    in1=e1[:, it0:it0 + B][:, :, None].to_broadcast(
                        [P, B, 2 * D]),
                    op=OP.add,
                )
                # score = where(s > 0, exp(s), exp(0.1*exp(s) - 0.1))
                t_t = pool.tile([P, M2], f32, tag="t")
                nc.scalar.activation(t_t[:], s2[:], ACTF.Exp,
                                     bias=zbias[:, 0:1], scale=1.0)
                u_t = pool.tile([P, M2], f32, tag="u")
                nc.scalar.activation(u_t[:], t_t[:], ACTF.Exp,
                                     bias=mbias[:, 0:1], scale=0.1)
                m_t = pool.tile([P, M2], mybir.dt.uint8, tag="m")
                nc.vector.tensor_scalar(
                    out=m_t[:], in0=s2[:], scalar1=0.0, scalar2=None,
                    op0=OP.is_gt,
                )
                nc.vector.copy_predicated(out=u_t[:], mask=m_t[:], data=t_t[:])
                # weights: score * per-parity validity mask
                w2t = pool.tile([P, M2], f32, tag="w2")
                nc.vector.tensor_tensor(
                    out=w2t[:], in0=u_t[:], in1=md_t[:, md_off:md_off + M2],
                    op=OP.mult,
                )
                # weighted rows -> packed [G | den] per iteration
                scaled = bigpool.tile([P, M2 * F], bf16, tag="scaled")
                nc.vector.tensor_tensor(
                    out=scaled[:].rearrange("p (m f) -> p m f", f=F),
                    in0=rows[:].rearrange("p (m f) -> p m f", f=F),
                    in1=w2t[:, :, None].to_broadcast([P, M2, F]),
                    op=OP.mult,
                )
                g65 = pool.tile([P, B * G1], f32, tag="g65")
                g65v = g65[:].rearrange("p (b w) -> p b w", w=G1)
                nc.vector.tensor_reduce(
                    out=g65v[:, :, 0:F],
                    in_=scaled[:].rearrange("p (b m f) -> p b f m",
                                            m=2 * D, f=F),
                    axis=AX.X, op=OP.add,
                )
                nc.vector.tensor_reduce(
                    out=den_all[:, it0:it0 + B],
                    in_=w2t[:].rearrange("p (b m) -> p b m", m=2 * D),
                    axis=AX.X, op=OP.add,
                )
                nc.vector.tensor_copy(
                    out=g65v[:, :, F], in_=den_all[:, it0:it0 + B],
                )
                # per-iteration: transpose [P, 65] and matmul vs [W; b]
                h_p = psum.tile([P, B * F], f32, tag="hp")
                for bb in range(B):
                    it = it0 + bb
                    gtp = psum.tile([G1, P], f32, tag="gtp")
                    nc.tensor.transpose(
                        out=gtp[:], in_=g65[:, bb * G1:(bb + 1) * G1],
                        identity=ident[:])
                    gts = pool.tile([G1, P], f32, tag="gts")
                    nc.vector.tensor_copy(out=gts[:], in_=gtp[:])
                    nc.tensor.matmul(
                        out=h_p[:, bb * F:(bb + 1) * F],
                        lhsT=gts[:],
                        rhs=wb65[:],
                        start=True, stop=True)
                nc.vector.tensor_copy(
                    out=h_all[:, it0 * F:(it0 + B) * F], in_=h_p[:],
                )
                it0 += B
                md_off += M2
                gi_off += B * S // 16

            # ---- final: divide by den_safe, write out --------------------
            m0 = pool.tile([P, NIT], f32, tag="m0")
            nc.vector.tensor_scalar(
                out=m0[:], in0=den_all[:], scalar1=0.0, scalar2=None,
                op0=OP.is_equal,
            )
            nc.vector.tensor_tensor(
                out=m0[:], in0=den_all[:], in1=m0[:], op=OP.add,
            )
            rec = pool.tile([P, NIT], f32, tag="rec")
            nc.vector.reciprocal(rec[:], m0[:])
            outs_t = bigpool.tile([P, NIT * F], f32, tag="outs")
            nc.vector.tensor_tensor(
                out=outs_t[:].rearrange("p (i f) -> p i f", f=F),
                in0=h_all[:].rearrange("p (i f) -> p i f", f=F),
                in1=rec[:, :, None].to_broadcast([P, NIT, F]),
                op=OP.mult,
            )
            nc.sync.dma_start(
                out=out3.rearrange("i p f -> p i f"),
                in_=outs_t[:].rearrange("p (i f) -> p i f", f=F),
            )


# ----------------------------------------------------------------------------
# host-side preparation (index plumbing only; host math is W @ a2 / b @ a2)
# ----------------------------------------------------------------------------

def prep_inputs(feature_a, feature_b, W, b, a_vec, edges, node_num_a,
                ncores=NCORES):
    fa = np.asarray(feature_a, np.float32)
    fb = np.asarray(feature_b, np.float32)
    W = np.asarray(W, np.float32)
    b = np.asarray(b, np.float32)
    a_vec = np.asarray(a_vec, np.float32).reshape(-1)
    edges = np.asarray(edges)
    na = int(node_num_a)
    assert na == NA and fb.shape == (NB, F) and fa.shape[1] == F

    a1 = a_vec[:F]
    a2 = a_vec[F:]
    Wa2 = (W @ a2).astype(np.float32)
    ba2 = float(b @ a2)

    src = edges[:, 0].astype(np.int64)
    dst = edges[:, 1].astype(np.int64)
    order = np.argsort(src, kind="stable")
    ssrc = src[order]
    sdst = dst[order]
    deg = np.bincount(ssrc, minlength=na).astype(np.int64)
    row_ptr = np.zeros(na + 1, np.int64)
    np.cumsum(deg, out=row_ptr[1:])

    # per-core degree-sorted batches of 128 nodes
    perms = []          # per core: node id per out row (or -1 for padding)
    Dmat = np.zeros((ncores, NIT), np.int64)
    for c in range(ncores):
        lo = c * NPC
        nodes = np.arange(lo, lo + NPC)
        p = nodes[np.argsort(deg[nodes], kind="stable")]
        p = np.concatenate([np.full(NROWS - NPC, -1, np.int64), p])
        # padding rows first (degree 0), keeps batches degree-sorted
        perms.append(p)
        dpad = np.concatenate([np.zeros(NROWS - NPC, np.int64), deg[p[NROWS - NPC:]]])
        Dmat[c] = dpad.reshape(NIT, P).max(axis=1)
    D_it = np.maximum(Dmat.max(axis=0), 1)       # shared widths across cores

    # group consecutive iterations of equal D (bounded group size)
    groups = []
    i = 0
    while i < NIT:
        j = i
        while j < NIT and D_it[j] == D_it[i] and (j - i) < 8 \
                and (j - i + 1) * D_it[i] <= 48:
            j += 1
        groups.append((int(D_it[i]), j - i))
        i = j

    MDW = int(sum(2 * D * B for D, B in groups))
    GW = int(sum(P * D * B // 16 for D, B in groups))

    in_maps = []
    for c in range(ncores):
        p = perms[c]
        pk_fa = np.zeros((P, NIT * F), np.float32)
        pk_md = np.zeros((P, MDW), np.float32)
        gidx = np.zeros((P, GW), np.int16)
        md_off = 0
        gi_off = 0
        it0 = 0
        for (D, B) in groups:
            S = P * D
            for bi in range(B):
                it = it0 + bi
                nid = p[it * P:(it + 1) * P]                 # [P]
                valid_n = nid >= 0
                nid_c = np.where(valid_n, nid, 0)
                pk_fa[:, it * F:(it + 1) * F] = np.where(
                    valid_n[:, None], fa[nid_c], 0.0)
                dg = np.where(valid_n, deg[nid_c], 0)         # [P]
                ks = np.arange(D)[None, :]                    # [1, D]
                vmask = ks < dg[:, None]                      # [P, D]
                pos = row_ptr[nid_c][:, None] + ks
                pos = np.clip(pos, 0, len(sdst) - 1)
                d_all = np.where(vmask, sdst[pos], 0)         # [P, D]
                odd = (d_all & 1).astype(bool)
                mask2 = np.zeros((P, D, 2), np.float32)
                mask2[:, :, 0] = (vmask & ~odd)
                mask2[:, :, 1] = (vmask & odd)
                o = md_off + 2 * D * bi
                pk_md[:, o:o + 2 * D] = mask2.reshape(P, 2 * D)
                idx = (d_all >> 1).astype(np.int16)           # [P, D]
                flat = idx.T.reshape(-1)                      # [(k p)] p-fastest
                sb = flat.reshape(S // 16, 16).T              # [16, S/16]
                go = gi_off + bi * S // 16
                gidx[:, go:go + S // 16] = np.tile(sb, (8, 1))
            it0 += B
            md_off += 2 * D * B
            gi_off += B * S // 16
        assert gidx.max() < 32768 and (NB - 1) >> 1 < 32768

        wvec = np.zeros((P, 3 * F), np.float32)
        wvec[:, 0:F] = a1[None, :]
        wvec[:, F:2 * F] = Wa2[None, :]
        wvec[:, 2 * F:3 * F] = b[None, :]
        in_maps.append(dict(
            fb_tab=np.ascontiguousarray(fb.reshape(NB // 2, 2 * F)),
            pk_fa=pk_fa,
            pk_md=pk_md,
            gidx=gidx,
            wvec=wvec,
            wmat=np.ascontiguousarray(W),
        ))

    cfg = dict(groups=groups, MDW=MDW, GW=GW, ba2=ba2, perms=perms)
    return in_maps, cfg


def build_bass(cfg, ncores=NCORES):
    nc = bacc.Bacc("TRN2", target_bir_lowering=False, debug=False,
                   enable_asserts=False, num_devices=ncores,
                   num_swdge_queues=NQ)
    ins = dict(
        fb_tab=nc.dram_tensor("fb_tab", [NB // 2, 2 * F], f32,
                              kind="ExternalInput").ap(),
        pk_fa=nc.dram_tensor("pk_fa", [P, NIT * F], f32,
                             kind="ExternalInput").ap(),
        pk_md=nc.dram_tensor("pk_md", [P, cfg["MDW"]], f32,
                             kind="ExternalInput").ap(),
        gidx=nc.dram_tensor("gidx", [P, cfg["GW"]], i16,
                            kind="ExternalInput").ap(),
        wvec=nc.dram_tensor("wvec", [P, 3 * F], f32, kind="ExternalInput").ap(),
        wmat=nc.dram_tensor("wmat", [F, F], f32, kind="ExternalInput").ap(),
    )
    outs = dict(
        out=nc.dram_tensor("out", [NROWS, F], f32, kind="ExternalOutput").ap(),
    )
    with tile.TileContext(nc) as tc:
        emit_program(tc, ins, outs, cfg)
    nc.compile()
    return nc


def assemble_output(results, cfg):
    full = np.zeros((NA, F), np.float32)
    for c in range(NCORES):
        p = cfg["perms"][c]
        rows = results[c]["out"]
        valid = p >= 0
        full[p[valid]] = rows[valid]
    return full


# ----------------------------------------------------------------------------
# entry points
# ----------------------------------------------------------------------------

def kernel_with_results(trace=False, **inputs):
    from concourse import bass_utils

    in_maps, cfg = prep_inputs(**inputs)
    nc = build_bass(cfg)
    res = bass_utils.run_bass_kernel_spmd(
        nc, in_maps, core_ids=list(range(NCORES)), trace=trace,
    )
    return assemble_output(res.results, cfg), res


def kernel(**inputs):
    return kernel_with_results(trace=False, **inputs)[0]


def kernel_timed(nreps=6, rep1=1, **inputs):
    """Reuses the compiled PJRT executable; times warm repeat executions with
    device-resident inputs.  Returns (out, [ns,...])."""
    import time
    import jax
    from jax.sharding import Mesh, PartitionSpec, NamedSharding
    from jax.experimental.shard_map import shard_map
    from concourse import bass2jax

    in_maps, cfg = prep_inputs(**inputs)
    cfg["rep1"] = rep1
    nc = build_bass(cfg)
    bass2jax.install_neuronx_cc_hook()

    ncores = NCORES
    partition_name = nc.partition_id_tensor.name if nc.partition_id_tensor else None
    in_names, out_names, out_avals, zero_outs = [], [], [], []
    for alloc in nc.m.functions[0].allocations:
        if not isinstance(alloc, mybir.MemoryLocationSet):
            continue
        name = alloc.memorylocations[0].name
        if alloc.kind == "ExternalInput":
            if name != partition_name:
                in_names.append(name)
        elif alloc.kind == "ExternalOutput":
            shape = tuple(alloc.tensor_shape)
            dtype = mybir.dt.np(alloc.dtype)
            out_avals.append(jax.core.ShapedArray(shape, dtype))
            out_names.append(name)
            zero_outs.append(np.zeros(shape, dtype))
    n_params = len(in_names)
    n_outs = len(out_avals)
    all_in_names = list(in_names) + list(out_names)
    if partition_name is not None:
        all_in_names.append(partition_name)

    def _body(*args):
        operands = list(args)
        if partition_name is not None:
            operands.append(bass2jax.partition_id_tensor())
        outs_ = bass2jax._bass_exec_p.bind(
            *operands,
            out_avals=tuple(out_avals),
            in_names=tuple(all_in_names),
            out_names=tuple(out_names),
            lowering_input_output_aliases=(),
            sim_require_finite=True,
            sim_require_nnan=True,
            nc=nc,
        )
        return tuple(outs_)

    devices = jax.devices()[:ncores]
    mesh = Mesh(np.asarray(devices), ("core",))
    spec = PartitionSpec("core")
    shard = NamedSharding(mesh, spec)
    sharded = jax.jit(
        shard_map(_body, mesh=mesh, in_specs=(spec,) * (n_params + n_outs),
                  out_specs=(spec,) * n_outs, check_rep=False),
        keep_unused=True,
    )
    concat_in = [
        np.concatenate([np.asarray(in_maps[c][nm]) for c in range(ncores)],
                       axis=0)
        for nm in in_names
    ]
    concat_zeros = [
        np.zeros((ncores * z.shape[0], *z.shape[1:]), z.dtype) for z in zero_outs
    ]
    dev_in = [jax.device_put(a, shard) for a in concat_in]
    dzs = [jax.device_put(z, shard) for z in concat_zeros]

    out_arrs = None
    times = []
    for rep in range(nreps + 1):
        t0 = time.perf_counter()
        res = sharded(*dev_in, *dzs)
        for r in res:
            r.block_until_ready()
        t1 = time.perf_counter()
        if rep > 0:
            times.append(int((t1 - t0) * 1e9))
        out_arrs = res

    results = []
    for c in range(ncores):
        m = {}
        for i, name in enumerate(out_names):
            m[name] = np.asarray(out_arrs[i]).reshape(
                ncores, *out_avals[i].shape)[c]
        results.append(m)
    return assemble_output(results, cfg), times


if __name__ == "__main__":
    np.random.seed(0)
    E = 800000
    ins = dict(
        feature_a=np.random.randn(NA, F).astype(np.float32),
        feature_b=np.random.randn(NB, F).astype(np.float32),
        W=(np.random.randn(F, F) / 8).astype(np.float32),
        b=np.zeros(F, np.float32),
        a_vec=(np.random.randn(2 * F, 1) * 0.05).astype(np.float32),
        edges=np.stack([np.random.randint(0, NA, E),
                        np.random.randint(0, NB, E)], 1).astype(np.int64),
        node_num_a=NA,
    )
    out = kernel(**ins)
    print(out.shape, out.dtype)
